# Initial kernel scaffold
#
import math
import os
import numpy as np
import ml_dtypes
import contextlib

import concourse.bass as bass
import concourse.tile as tile
from concourse import bacc, mybir, masks
from concourse.bass_utils import run_bass_kernel_spmd

F32 = mybir.dt.float32
F32R = mybir.dt.float32r
BF16 = mybir.dt.bfloat16
F16 = mybir.dt.float16
ALU = mybir.AluOpType
AF = mybir.ActivationFunctionType
AX = mybir.AxisListType

NCORES = 8
FRAME = 5
NF = FRAME - 1
D = 512
DH = 64
B = 20
NPTS = 1024
BE = B // FRAME
NPC = NPTS // NCORES     # 128 points per core
TOK = NF * BE * NPC      # 2048 tokens per core
NLAYER = 12
LNEPS = 1e-5
BNEPS = 1e-5
NBN = 16 * NPTS

CONV_DIMS = [2048, 1536, 1024, 768, 512]


def _pe_table(max_len=16, d=D):
    pos = np.arange(max_len, dtype=np.float32)[:, None]
    div = np.exp(np.arange(0, d, 2, dtype=np.float32) * (-math.log(10000.0) / d))
    pe = np.zeros((max_len, d), np.float32)
    pe[:, 0::2] = np.sin(pos * div)
    pe[:, 1::2] = np.cos(pos * div)
    return pe


def build_kernel():
    nc = bacc.Bacc("TRN2", target_bir_lowering=False, debug=False,
                   num_devices=NCORES)

    xin = nc.dram_tensor("xin", [CONV_DIMS[0], TOK], F16, kind="ExternalInput").ap()
    convw = [nc.dram_tensor(f"convw{i}", [CONV_DIMS[i], CONV_DIMS[i + 1]], F16,
                            kind="ExternalInput").ap() for i in range(4)]
    bnconst = [nc.dram_tensor(f"bnconst{i}", [128, 3 * (CONV_DIMS[i + 1] // 128)],
                              F32, kind="ExternalInput").ap() for i in range(3)]
    bias4 = nc.dram_tensor("bias4", [128, 4 * NF], F32, kind="ExternalInput").ap()

    wqkv_d = nc.dram_tensor("wqkv", [NLAYER, D, 3 * D], F32R, kind="ExternalInput").ap()
    bqkv_d = nc.dram_tensor("bqkv", [NLAYER, 1, 3 * D], BF16, kind="ExternalInput").ap()
    wsum_d = nc.dram_tensor("wsum", [NLAYER, 1, 3 * D], BF16, kind="ExternalInput").ap()
    wo_d = nc.dram_tensor("wo", [NLAYER, D, D], BF16, kind="ExternalInput").ap()
    w1_d = nc.dram_tensor("w1", [NLAYER, D, D], F32R, kind="ExternalInput").ap()
    w2_d = nc.dram_tensor("w2", [NLAYER, D, D], F32R, kind="ExternalInput").ap()
    tcols_d = nc.dram_tensor("tcols", [NLAYER, 128, 12], F32, kind="ExternalInput").ap()

    projw_d = nc.dram_tensor("projw", [D, D], F32R, kind="ExternalInput").ap()
    rw1_d = nc.dram_tensor("rw1", [D, 256], F32R, kind="ExternalInput").ap()
    rw2_d = nc.dram_tensor("rw2", [256, 128], F32R, kind="ExternalInput").ap()
    rw3_d = nc.dram_tensor("rw3", [128, 8], F32R, kind="ExternalInput").ap()
    tw1_d = nc.dram_tensor("tw1", [D, 256], F32R, kind="ExternalInput").ap()
    tw2_d = nc.dram_tensor("tw2", [256, 128], F32R, kind="ExternalInput").ap()
    tw3_d = nc.dram_tensor("tw3", [128, 8], F32R, kind="ExternalInput").ap()
    hcols_d = nc.dram_tensor("hcols", [128, 10], F32, kind="ExternalInput").ap()
    sb3_d = nc.dram_tensor("sb3", [8, 2], F32, kind="ExternalInput").ap()
    ones_d = nc.dram_tensor("ones_c", [128, 128], F32, kind="ExternalInput").ap()

    d6_o = nc.dram_tensor("d6", [8, TOK], F32, kind="ExternalOutput").ap()
    tr_o = nc.dram_tensor("tr3", [8, TOK], F32, kind="ExternalOutput").ap()
    xdbg_o = None
    if os.environ.get("KHEADS", "1") == "0":
        xdbg_o = nc.dram_tensor("xdbg", [8, 128, TOK], F32,
                                kind="ExternalOutput").ap()

    with tile.TileContext(nc) as tc, contextlib.ExitStack() as ctx:
        const_p = ctx.enter_context(tc.tile_pool(name="consts", bufs=1))
        onescol = const_p.tile([128, 1], F32R)
        onesrow = const_p.tile([1, 128], F32R)
        ident = const_p.tile([128, 128], BF16)
        nc.gpsimd.dma_start(onescol[:], ones_d[:, 0:1])
        nc.gpsimd.dma_start(onesrow[:], ones_d[0:1, :])
        masks.make_identity(nc, ident[:])

        xs_p = ctx.enter_context(tc.tile_pool(name="xstate", bufs=1))
        xA = [xs_p.tile([128, TOK], F32R, tag=f"xA{m}", name=f"xA{m}") for m in range(4)]
        xB = [xs_p.tile([128, TOK], F32R, tag=f"xB{m}", name=f"xB{m}") for m in range(4)]

        rows_p = ctx.enter_context(tc.tile_pool(name="rows", bufs=1))
        stat_p = ctx.enter_context(tc.tile_pool(name="stats", bufs=1))
        scr = ctx.enter_context(tc.tile_pool(name="scratch", bufs=2))
        dram_p = ctx.enter_context(tc.tile_pool(name="dramb", bufs=1, space="DRAM"))

        pp_mm = ctx.enter_context(tc.tile_pool(name="ppmm", bufs=4, space="PSUM"))
        pp_row = ctx.enter_context(tc.tile_pool(name="pprow", bufs=1, space="PSUM"))
        pp_bc = ctx.enter_context(tc.tile_pool(name="ppbc", bufs=2, space="PSUM"))

        y_dram = [dram_p.tile([CONV_DIMS[i], TOK], F16, tag=f"ydram{i}", name=f"ydram{i}")
                  for i in range(1, 4)]

        # ------------------------------------------------------------------
        # conv stack (activations spilled to DRAM, BN applied on load)
        # ------------------------------------------------------------------
        b4sb = stat_p.tile([128, 4 * NF], F32, tag="b4")
        nc.sync.dma_start(b4sb[:], bias4[:])

        bn_s = {}
        bn_t = {}

        def conv_layer(li, wcp, cxp, pp_conv):
            kdim, mdim = CONV_DIMS[li - 1], CONV_DIMS[li]
            KC, MC = kdim // 128, mdim // 128
            src = xin if li == 1 else y_dram[li - 2]
            with_bn = li < 4
            if with_bn:
                sum_acc = stat_p.tile([128, MC * 4], F32, tag=f"sum{li}")
                sq_acc = stat_p.tile([128, MC * 4], F32, tag=f"sq{li}")
            # whole layer's weights resident, loaded once (not per pt chunk);
            # single shared allocation reused across conv layers
            wall = wcp.tile([128, CONV_DIMS[0] // 128 * 12 * 128], F16,
                            tag="wall", name=f"wall{li}")
            for m in range(MC):
                nc.sync.dma_start(
                    wall[:, m * KC * 128:(m + 1) * KC * 128]
                    .rearrange("p (k c) -> p k c", k=KC),
                    convw[li - 1][:, m * 128:(m + 1) * 128]
                    .rearrange("(k p) c -> p k c", p=128))
            for pt in range(4):
                xt = cxp.tile([128, KC * 512], F16, tag="convx", name="convx",
                              bufs=2)
                nc.sync.dma_start(
                    xt[:].rearrange("p (k c) -> p k c", k=KC),
                    src[:, pt * 512:(pt + 1) * 512]
                    .rearrange("(k p) c -> p k c", p=128))
                if li > 1:
                    s_p, t_p = bn_s[li - 1], bn_t[li - 1]
                    for k in range(KC):
                        nc.scalar.activation(
                            xt[:, k * 512:(k + 1) * 512],
                            xt[:, k * 512:(k + 1) * 512],
                            AF.Relu, bias=t_p[:, k:k + 1], scale=s_p[:, k:k + 1])
                for m in range(MC):
                    ps = pp_conv.tile([128, 512], F32, tag="mm", name="cps")
                    for k in range(KC):
                        nc.tensor.matmul(
                            ps[:], wall[:, (m * KC + k) * 128:(m * KC + k + 1) * 128],
                            xt[:, k * 512:(k + 1) * 512],
                            start=(k == 0), stop=(k == KC - 1))
                    if with_bn:
                        ot = cxp.tile([128, 512], F16, tag="convot", name="cot",
                                      bufs=3)
                        nc.scalar.activation(
                            ot[:], ps[:], AF.Copy,
                            accum_out=sum_acc[:, m * 4 + pt:m * 4 + pt + 1])
                        sqs = cxp.tile([128, 512], F16, tag="sqscr", name="sqs",
                                       bufs=3)
                        nc.scalar.activation(
                            sqs[:], ps[:], AF.Square,
                            accum_out=sq_acc[:, m * 4 + pt:m * 4 + pt + 1])
                        nc.sync.dma_start(
                            y_dram[li - 1][m * 128:(m + 1) * 128,
                                           pt * 512:(pt + 1) * 512], ot[:])
                    else:
                        nc.scalar.activation(
                            xA[m][:, pt * 512:(pt + 1) * 512], ps[:], AF.Identity,
                            bias=b4sb[:, m * 4 + pt:m * 4 + pt + 1])
            if not with_bn:
                return
            allin = stat_p.tile([128, 2 * MC], F32, tag=f"ain{li}", name="allin")
            nc.vector.tensor_reduce(
                allin[:, 0:MC], sum_acc[:].rearrange("p (m t) -> p m t", m=MC),
                axis=AX.X, op=ALU.add)
            nc.vector.tensor_reduce(
                allin[:, MC:2 * MC], sq_acc[:].rearrange("p (m t) -> p m t", m=MC),
                axis=AX.X, op=ALU.add)
            bin_ = dram_p.tile([128, 2 * MC], F32, tag=f"arin{li}", name="arin")
            bout = dram_p.tile([128, 2 * MC], F32, tag=f"arout{li}", name="arout")
            nc.sync.dma_start(bin_[:], allin[:])
            nc.gpsimd.collective_compute(
                "AllReduce", ALU.add, replica_groups=[list(range(NCORES))],
                ins=[bin_.opt()], outs=[bout.opt()])
            gl = stat_p.tile([128, 2 * MC], F32, tag=f"gl{li}", name="gl")
            nc.sync.dma_start(gl[:], bout[:])
            cst = stat_p.tile([128, 3 * MC], F32, tag=f"cst{li}", name="cst")
            nc.sync.dma_start(cst[:], bnconst[li - 1][:])
            mu = stat_p.tile([128, MC], F32, tag=f"mu{li}", name="bmu")
            var = stat_p.tile([128, MC], F32, tag=f"va{li}", name="bvar")
            s_t = stat_p.tile([128, MC], F32, tag=f"s{li}", name="bs")
            t_t = stat_p.tile([128, MC], F32, tag=f"t{li}", name="bt")
            nc.scalar.mul(mu[:], gl[:, 0:MC], 1.0 / NBN)
            nc.scalar.mul(var[:], gl[:, MC:2 * MC], 1.0 / NBN)
            msq = stat_p.tile([128, MC], F32, tag=f"ms{li}", name="bmsq")
            nc.vector.tensor_mul(msq[:], mu[:], mu[:])
            nc.vector.tensor_tensor(var[:], var[:], msq[:], op=ALU.subtract)
            nc.vector.tensor_scalar(var[:], var[:], BNEPS, None, op0=ALU.add)
            sd = stat_p.tile([128, MC], F32, tag=f"sd{li}", name="bsd")
            nc.scalar.activation(sd[:], var[:], AF.Sqrt)
            rsd = stat_p.tile([128, MC], F32, tag=f"rs{li}", name="brsd")
            nc.vector.reciprocal(rsd[:], sd[:])
            nc.vector.tensor_mul(s_t[:], rsd[:], cst[:, 0:MC])
            nc.vector.tensor_mul(t_t[:], mu[:], s_t[:])
            nc.vector.tensor_tensor(t_t[:], cst[:, MC:2 * MC], t_t[:],
                                    op=ALU.subtract)
            bn_s[li], bn_t[li] = s_t, t_t

        with tc.tile_pool(name="wcp", bufs=1) as wcp, \
             tc.tile_pool(name="cxp", bufs=1) as cxp:
            for li in (1, 2, 3, 4):
                conv_layer(li, wcp, cxp, pp_mm)

        # ------------------------------------------------------------------
        # transformer
        # ------------------------------------------------------------------
        def ln_cols(xt, xview, dst_tiles, dst_cols):
            """LN per token over feature dim (stats + apply on DVE)."""
            ps_s = pp_row.tile([1, 512], F32, tag="row_s", name="ps_s")
            ps_q = pp_row.tile([1, 512], F32, tag="row_q", name="ps_q")
            for k in range(4):
                nc.tensor.matmul(ps_s[:], onescol[:], xview(k),
                                 start=(k == 0), stop=(k == 3))
            for k in range(4):
                sq = scr.tile([128, 512], F32R, tag="lnsq", name="lnsq")
                nc.scalar.square(sq[:], xview(k))
                nc.tensor.matmul(ps_q[:], onescol[:], sq[:],
                                 start=(k == 0), stop=(k == 3))
            mu = rows_p.tile([1, 512], F32R, tag="mu", name="lmu", bufs=2)
            e2 = rows_p.tile([1, 512], F32, tag="e2", name="le2", bufs=2)
            r = rows_p.tile([1, 512], F32R, tag="r", name="lr", bufs=2)
            nc.scalar.mul(mu[:], ps_s[:], 1.0 / D)
            nc.scalar.mul(e2[:], ps_q[:], 1.0 / D)
            with nc.allow_low_precision(reason="f32r row math"):
                nc.vector.tensor_mul(r[:], mu[:], mu[:])
                nc.vector.tensor_tensor(e2[:], e2[:], r[:], op=ALU.subtract)
                nc.vector.tensor_scalar(e2[:], e2[:], LNEPS, None, op0=ALU.add)
                nc.scalar.activation(e2[:], e2[:], AF.Sqrt)
                nc.vector.reciprocal(r[:], e2[:])
            psb_mu = pp_bc.tile([128, 512], F32, tag="bc", name="psbmu")
            psb_r = pp_bc.tile([128, 512], F32, tag="bc", name="psbr")
            nc.tensor.matmul(psb_mu[:], onesrow[:], mu[:], start=True, stop=True)
            nc.tensor.matmul(psb_r[:], onesrow[:], r[:], start=True, stop=True)
            for k in range(4):
                tmp = scr.tile([128, 512], F32, tag="lntmp", name="lntmp")
                nc.vector.tensor_tensor(tmp[:], xview(k), psb_mu[:],
                                        op=ALU.subtract)
                nc.vector.tensor_mul(dst_tiles[k][:, dst_cols], tmp[:], psb_r[:])

        def ln1_rows(x_in, negmu_all, sd_all, rcol_all):
            """Per-frame LN stats; rows for the qkv matmul fold."""
            for f in range(4):
                sl = slice(f * 512, (f + 1) * 512)
                ps_s = pp_row.tile([1, 512], F32, tag="row_s", name="ps_s")
                ps_q = pp_row.tile([1, 512], F32, tag="row_q", name="ps_q")
                for k in range(4):
                    nc.tensor.matmul(ps_s[:], onescol[:], x_in[k][:, sl],
                                     start=(k == 0), stop=(k == 3))
                for k in range(4):
                    sq = scr.tile([128, 512], F32R, tag="lnsq", name="lnsq")
                    nc.scalar.square(sq[:], x_in[k][:, sl])
                    nc.tensor.matmul(ps_q[:], onescol[:], sq[:],
                                     start=(k == 0), stop=(k == 3))
                mu = rows_p.tile([1, 512], F32, tag="mu", name="lmu", bufs=2)
                e2 = rows_p.tile([1, 512], F32, tag="e2", name="le2", bufs=2)
                rr = rows_p.tile([1, 512], F32, tag="rr", name="lrr", bufs=2)
                nc.scalar.mul(mu[:], ps_s[:], 1.0 / D)
                nc.scalar.mul(e2[:], ps_q[:], 1.0 / D)
                with nc.allow_low_precision(reason="ln1 rows"):
                    nc.scalar.mul(negmu_all[:, sl], ps_s[:], -1.0 / D)
                    msq = rows_p.tile([1, 512], F32, tag="rr", name="lms", bufs=2)
                    nc.vector.tensor_mul(msq[:], mu[:], mu[:])
                    nc.vector.tensor_tensor(e2[:], e2[:], msq[:], op=ALU.subtract)
                    nc.vector.tensor_scalar(e2[:], e2[:], LNEPS, None, op0=ALU.add)
                    nc.scalar.activation(sd_all[:, sl], e2[:], AF.Sqrt)
                    nc.scalar.activation(e2[:], e2[:], AF.Sqrt)
                    nc.vector.reciprocal(rr[:], e2[:])
                for st in range(4):
                    nc.sync.dma_start(
                        rcol_all[:, f * 4 + st:f * 4 + st + 1],
                        rr[0:1, st * 128:(st + 1) * 128])

        tr_ctx = ctx.enter_context(contextlib.ExitStack())
        wp = tr_ctx.enter_context(tc.tile_pool(name="wp", bufs=1))
        wqp = tr_ctx.enter_context(tc.tile_pool(name="wqp", bufs=1))
        attn_p = tr_ctx.enter_context(tc.tile_pool(name="attn", bufs=2))
        sl_p = tr_ctx.enter_context(tc.tile_pool(name="slices", bufs=1))
        ot_p = tr_ctx.enter_context(tc.tile_pool(name="otp", bufs=1))
        otb = [ot_p.tile([128, TOK], BF16, tag=f"ot{m}", name=f"otb{m}")
               for m in range(4)]

        def transformer_layer(li, x_in, x_mid):
            wq = [wqp.tile([128, 3 * D], F32R, tag=f"wqkv{k}", name=f"wq{k}")
                  for k in range(4)]
            for k in range(4):
                nc.sync.dma_start(wq[k][:], wqkv_d[li, k * 128:(k + 1) * 128, :])
            bq = rows_p.tile([1, 3 * D], BF16, tag="bqkv", name="bq", bufs=1)
            nc.sync.dma_start(bq[:], bqkv_d[li])
            ws = rows_p.tile([1, 3 * D], BF16, tag="wsum", name="ws", bufs=1)
            nc.sync.dma_start(ws[:], wsum_d[li])
            cols = stat_p.tile([128, 12], F32, tag="tcols", name="tcols")
            nc.sync.dma_start(cols[:], tcols_d[li])

            negmu = attn_p.tile([1, TOK], BF16, tag="negmu", name="negmu", bufs=1)
            sd_all = attn_p.tile([1, TOK], BF16, tag="sdall", name="sdall", bufs=1)
            rcol = attn_p.tile([128, 16], F32, tag="rcol", name="rcol", bufs=1)
            ln1_rows(x_in, negmu, sd_all, rcol)

            for st in range(4):
                qt = attn_p.tile([128, TOK], BF16, tag="qst", name="qt", bufs=1)
                kt = attn_p.tile([128, TOK], BF16, tag="kst", name="kt", bufs=1)
                vt = attn_p.tile([128, TOK], BF16, tag="vst", name="vt", bufs=1)
                qkv_dst = [qt, kt, vt]
                for f in range(NF):
                    c0 = f * 512 + st * 128
                    for ns in range(3):
                        ps = pp_mm.tile([128, 512], F32, tag="mm", name="qps")
                        for k in range(4):
                            nc.tensor.matmul(
                                ps[:], x_in[k][:, c0:c0 + 128],
                                wq[k][:, ns * 512:(ns + 1) * 512],
                                start=(k == 0), stop=False)
                        nc.tensor.matmul(ps[:], negmu[0:1, c0:c0 + 128],
                                         ws[:, ns * 512:(ns + 1) * 512],
                                         start=False, stop=False)
                        nc.tensor.matmul(ps[:], sd_all[0:1, c0:c0 + 128],
                                         bq[:, ns * 512:(ns + 1) * 512],
                                         start=False, stop=True)
                        nc.scalar.activation(
                            qkv_dst[ns][:, f * 512:(f + 1) * 512], ps[:],
                            AF.Copy, scale=rcol[:, f * 4 + st:f * 4 + st + 1])

                s_sc = attn_p.tile([128, 128], F32, tag="s_sc", name="s_sc",
                                   bufs=1)
                k4 = kt[:].rearrange("p (j hd) -> p j hd", j=4)
                for i in range(4):
                    pbig = attn_p.tile([128, TOK], BF16, tag="pbig", name="pbig", bufs=1)
                    qi = qt[:, i * 512:(i + 1) * 512].unsqueeze(1) \
                        .broadcast_to([128, 4, 512])
                    nc.vector.tensor_mul(
                        pbig[:].rearrange("p (j hd) -> p j hd", j=4), qi, k4)
                    nc.vector.tensor_reduce(
                        s_sc[:, i * 32:(i + 1) * 32],
                        pbig[:].rearrange("p (g d) -> p g d", g=32),
                        axis=AX.X, op=ALU.add)
                # softmax over j without max-subtraction (logits bounded)
                # S cols = i*32 + j*8 + h -> exp -> A cols = i*32 + h*4 + j
                eexp = attn_p.tile([128, 128], BF16, tag="eexp", name="eexp",
                                   bufs=1)
                nc.scalar.activation(
                    eexp[:].rearrange("p (i h j) -> p i h j", i=4, h=8),
                    s_sc[:].rearrange("p (i j h) -> p i h j", i=4, j=4), AF.Exp)
                z = attn_p.tile([128, 32], F32, tag="z", name="zt", bufs=1)
                nc.vector.tensor_reduce(
                    z[:].rearrange("p (i h) -> p i h", i=4),
                    eexp[:].rearrange("p (i h j) -> p i h j", i=4, h=8),
                    axis=AX.X, op=ALU.add)
                zr = attn_p.tile([128, 32], F32, tag="zr", name="zr", bufs=1)
                nc.vector.reciprocal(zr[:], z[:])
                a_t = attn_p.tile([128, 128], BF16, tag="a_t", name="a_t",
                                  bufs=1)
                nc.vector.tensor_mul(
                    a_t[:].rearrange("p (i h j) -> p i h j", i=4, h=8),
                    eexp[:].rearrange("p (i h j) -> p i h j", i=4, h=8),
                    zr[:].rearrange("p (i h) -> p i h", i=4).unsqueeze(3)
                    .broadcast_to([128, 4, 8, 4]))
                v4 = vt[:].rearrange("p (j h d) -> p j h d", j=4, h=8)
                for i in range(4):
                    tbig = attn_p.tile([128, TOK], BF16, tag="tbig", name="tbig", bufs=1)
                    ablk = a_t[:, i * 32:(i + 1) * 32] \
                        .rearrange("p (h j) -> p j h", h=8) \
                        .unsqueeze(3).broadcast_to([128, 4, 8, 64])
                    nc.vector.tensor_mul(
                        tbig[:].rearrange("p (j h d) -> p j h d", j=4, h=8),
                        v4, ablk)
                    of32 = attn_p.tile([128, 512], F32, tag="of32", name="of32",
                                       bufs=1)
                    nc.vector.tensor_reduce(
                        of32[:].rearrange("p (h d) -> p h d", h=8),
                        tbig[:].rearrange("p (j h d) -> p h d j", j=4, h=8),
                        axis=AX.X, op=ALU.add)
                    obf = attn_p.tile([128, 512], BF16, tag="obf", name="obf",
                                      bufs=1)
                    nc.scalar.copy(obf[:], of32[:])
                    for c in range(4):
                        pst = pp_bc.tile([128, 128], BF16, tag="bc", name="pst")
                        nc.tensor.transpose(pst[:], obf[:, c * 128:(c + 1) * 128],
                                            ident[:])
                        nc.scalar.copy(
                            otb[c][:, i * 512 + st * 128:i * 512 + st * 128 + 128],
                            pst[:])

            wo = [wp.tile([128, D], BF16, tag=f"wo{k}", name=f"wo{k}")
                  for k in range(4)]
            for k in range(4):
                nc.sync.dma_start(wo[k][:], wo_d[li, k * 128:(k + 1) * 128, :])
            for m in range(4):
                for ns in range(4):
                    ps = pp_mm.tile([128, 512], F32, tag="mm", name="ops")
                    for k in range(4):
                        nc.tensor.matmul(
                            ps[:], wo[k][:, m * 128:(m + 1) * 128],
                            otb[k][:, ns * 512:(ns + 1) * 512],
                            start=(k == 0), stop=(k == 3))
                    nc.vector.scalar_tensor_tensor(
                        x_mid[m][:, ns * 512:(ns + 1) * 512], ps[:],
                        cols[:, 4 + m:5 + m], x_in[m][:, ns * 512:(ns + 1) * 512],
                        op0=ALU.add, op1=ALU.add)

            w1 = [wp.tile([128, D], F32R, tag=f"w1_{k}", name=f"w1_{k}")
                  for k in range(4)]
            w2 = [wp.tile([128, D], F32R, tag=f"w2_{k}", name=f"w2_{k}")
                  for k in range(4)]
            for k in range(4):
                nc.sync.dma_start(w1[k][:], w1_d[li, k * 128:(k + 1) * 128, :])
                nc.sync.dma_start(w2[k][:], w2_d[li, k * 128:(k + 1) * 128, :])
            for ns in range(4):
                xh2 = [sl_p.tile([128, 512], F32R, tag=f"xh2_{k}", name=f"xh2_{k}")
                       for k in range(4)]
                ln_cols(x_mid,
                        lambda k: x_mid[k][:, ns * 512:(ns + 1) * 512],
                        xh2, slice(0, 512))
                hsl = [sl_p.tile([128, 512], F32R, tag=f"h_{m}", name=f"hsl{m}")
                       for m in range(4)]
                for m in range(4):
                    ps = pp_mm.tile([128, 512], F32, tag="mm", name="m1ps")
                    for k in range(4):
                        nc.tensor.matmul(
                            ps[:], w1[k][:, m * 128:(m + 1) * 128], xh2[k][:],
                            start=(k == 0), stop=(k == 3))
                    nc.scalar.activation(hsl[m][:], ps[:], AF.Gelu_apprx_tanh,
                                         bias=cols[:, m:m + 1])
                for m in range(4):
                    ps = pp_mm.tile([128, 512], F32, tag="mm", name="m2ps")
                    for k in range(4):
                        nc.tensor.matmul(
                            ps[:], w2[k][:, m * 128:(m + 1) * 128], hsl[k][:],
                            start=(k == 0), stop=(k == 3))
                    nc.vector.scalar_tensor_tensor(
                        x_mid[m][:, ns * 512:(ns + 1) * 512], ps[:],
                        cols[:, 8 + m:9 + m], x_mid[m][:, ns * 512:(ns + 1) * 512],
                        op0=ALU.add, op1=ALU.add)

        cur, alt = xA, xB
        _nl = int(os.environ.get("KNLAYERS", NLAYER))
        _bar = os.environ.get("KBAR", "0") == "1"
        for li in range(_nl):
            transformer_layer(li, cur, alt)
            cur, alt = alt, cur
            if _bar:
                nc.all_engine_barrier()

        tr_ctx.close()

        # ------------------------------------------------------------------
        # heads
        # ------------------------------------------------------------------
        _skip_heads = os.environ.get("KHEADS", "1") == "0"
        if _skip_heads:
            _da, _db = (int(c) for c in os.environ.get("KDUMP", "01"))
            nc.gpsimd.dma_start(d6_o[:], cur[_da][0:8, :])
            nc.gpsimd.dma_start(tr_o[:], cur[_db][0:8, :])
            for m in range(4):
                nc.gpsimd.dma_start(xdbg_o[m], cur[m][:])
                nc.gpsimd.dma_start(xdbg_o[4 + m], alt[m][:])
        with tc.tile_pool(name="heads", bufs=1) as hp, \
             tc.tile_pool(name="whp", bufs=1) as whp:
          if not _skip_heads:
              hc = stat_p.tile([128, 10], F32, tag="hcols", name="hc")
              nc.sync.dma_start(hc[:], hcols_d[:])
              sb3 = stat_p.tile([8, 2], F32, tag="sb3", name="sb3")
              nc.sync.dma_start(sb3[:], sb3_d[:])

              xhf = [hp.tile([128, TOK], F32R, tag=f"xhf{k}", name=f"xhf{k}")
                     for k in range(4)]
              for st in range(4):
                  ln_cols(cur, lambda k: cur[k][:, st * 512:(st + 1) * 512],
                          xhf, slice(st * 512, (st + 1) * 512))

              xp = alt

              def mm_head(src_tiles, wt_dram, kdim, mdim, dst_tiles, evict):
                  KC = kdim // 128
                  MC = max(mdim // 128, 1)
                  wsb = [whp.tile([128, mdim], F32R, tag=f"wh_{kdim}_{mdim}_{k}",
                                  name=f"wh{k}") for k in range(KC)]
                  for k in range(KC):
                      nc.sync.dma_start(wsb[k][:], wt_dram[k * 128:(k + 1) * 128, :])
                  for m in range(MC):
                      for ns in range(4):
                          ps = pp_mm.tile([128, 512], F32, tag="mm", name="hps")
                          for k in range(KC):
                              nc.tensor.matmul(
                                  ps[:], wsb[k][:, m * 128:(m + 1) * 128],
                                  src_tiles[k][:, ns * 512:(ns + 1) * 512],
                                  start=(k == 0), stop=(k == KC - 1))
                          evict(ps, dst_tiles[m], m, ns)

              mm_head(xhf, projw_d, D, D, xp,
                      lambda ps, dst, m, ns: nc.scalar.activation(
                          dst[:, ns * 512:(ns + 1) * 512], ps[:], AF.Identity,
                          bias=hc[:, m:m + 1]))

              def branch(w1d, w2d, w3d, b1ofs, b2ofs, out_dram, b3col, r1, r2, pfx):
                  mm_head(xp, w1d, D, 256, r1,
                          lambda ps, dst, m, ns: nc.scalar.activation(
                              dst[:, ns * 512:(ns + 1) * 512], ps[:], AF.Relu,
                              bias=hc[:, b1ofs + m:b1ofs + m + 1]))
                  mm_head(r1, w2d, 256, 128, r2,
                          lambda ps, dst, m, ns: nc.scalar.activation(
                              dst[:, ns * 512:(ns + 1) * 512], ps[:], AF.Relu,
                              bias=hc[:, b2ofs:b2ofs + 1]))
                  w3 = whp.tile([128, 8], F32R, tag=f"w3{pfx}", name="w3")
                  nc.sync.dma_start(w3[:], w3d[:])
                  out_sb = hp.tile([8, TOK], F32, tag=f"{pfx}out", name=f"{pfx}out")
                  for ns in range(4):
                      ps = pp_mm.tile([8, 512], F32, tag="mm", name="bps")
                      nc.tensor.matmul(ps[:], w3[:], r2[0][:, ns * 512:(ns + 1) * 512],
                                       start=True, stop=True)
                      nc.scalar.activation(out_sb[:, ns * 512:(ns + 1) * 512], ps[:],
                                           AF.Identity, bias=b3col)
                  nc.sync.dma_start(out_dram[:], out_sb[:])

              # reuse dead transformer buffers for intermediates
              branch(rw1_d, rw2_d, rw3_d, 4, 8, d6_o, sb3[:, 0:1],
                     [cur[0], cur[1]], [xhf[0]], "r")
              branch(tw1_d, tw2_d, tw3_d, 6, 9, tr_o, sb3[:, 1:2],
                     [cur[2], cur[3]], [xhf[1]], "t")

    nc.compile()
    return nc


# ----------------------------------------------------------------------------
# host side
# ----------------------------------------------------------------------------

_CACHE = {}


def _normalize_np(v, eps=1e-12):
    return v / np.maximum(np.linalg.norm(v, axis=-1, keepdims=True), eps)


def _rot6d_np(d6):
    a1, a2 = d6[..., :3], d6[..., 3:]
    b1 = _normalize_np(a1)
    b2 = _normalize_np(a2 - np.sum(b1 * a2, -1, keepdims=True) * b1)
    b3 = np.cross(b1, b2)
    return np.stack([b1, b2, b3], axis=-2)


def _prep_weights(inp):
    f32 = np.float32
    wmap = {}
    for i, cw in enumerate(['c1w', 'c2w', 'c3w', 'c4w']):
        wmap[f'convw{i}'] = np.ascontiguousarray(inp[cw].T.astype(np.float16))
    for i, (g, b2) in enumerate([('bn1g', 'bn1b'), ('bn2g', 'bn2b'),
                                 ('bn3g', 'bn3b')]):
        M = CONV_DIMS[i + 1] // 128
        bn = np.concatenate([
            inp[g].reshape(M, 128).T, inp[b2].reshape(M, 128).T,
            inp[f'c{i + 1}b'].reshape(M, 128).T], axis=1)
        wmap[f'bnconst{i}'] = np.ascontiguousarray(bn.astype(f32))
    pe = _pe_table()[:NF]
    b4 = inp['c4b'][None, :].astype(f32) + pe                   # (4, 512)
    # cols: m*4 + pt ; frame index == pt
    wmap['bias4'] = np.ascontiguousarray(
        b4.reshape(NF, 4, 128).transpose(2, 1, 0).reshape(128, 16).astype(f32))

    qkvw = np.array(inp['qkvw'], f32)
    qkvb = np.array(inp['qkvb'], f32)
    qkvw[:, :, :512] /= math.sqrt(DH)
    qkvb[:, :512] /= math.sqrt(DH)
    g1 = np.array(inp['ln1g'], f32)
    b1 = np.array(inp['ln1b'], f32)
    wq_fold = g1[:, :, None] * qkvw
    bq_fold = qkvb + np.einsum('ld,ldf->lf', b1, qkvw)
    wmap['wqkv'] = np.ascontiguousarray(wq_fold.astype(f32))
    wmap['bqkv'] = np.ascontiguousarray(bq_fold[:, None, :].astype(ml_dtypes.bfloat16))
    wmap['wsum'] = np.ascontiguousarray(wq_fold.sum(axis=1, keepdims=True)
                                        .astype(ml_dtypes.bfloat16))
    wmap['wo'] = np.ascontiguousarray(np.array(inp['outw'], f32)
                                      .astype(ml_dtypes.bfloat16))
    g2 = np.array(inp['ln2g'], f32)
    bl2 = np.array(inp['ln2b'], f32)
    m1w = np.array(inp['m1w'], f32)
    w1_fold = g2[:, :, None] * m1w
    b1_fold = np.array(inp['m1b'], f32) + np.einsum('ld,ldf->lf', bl2, m1w)
    wmap['w1'] = np.ascontiguousarray(w1_fold.astype(f32))
    wmap['w2'] = np.ascontiguousarray(np.array(inp['m2w'], f32))
    cols = np.zeros((NLAYER, 128, 12), f32)
    cols[:, :, 0:4] = b1_fold.reshape(NLAYER, 4, 128).transpose(0, 2, 1)
    cols[:, :, 4:8] = np.array(inp['outb'], f32).reshape(NLAYER, 4, 128) \
        .transpose(0, 2, 1)
    cols[:, :, 8:12] = np.array(inp['m2b'], f32).reshape(NLAYER, 4, 128) \
        .transpose(0, 2, 1)
    wmap['tcols'] = cols

    gf_ = np.array(inp['lnfg'], f32)
    bf_ = np.array(inp['lnfb'], f32)
    projw = np.array(inp['projw'], f32)
    wmap['projw'] = np.ascontiguousarray(gf_[:, None] * projw)
    projb_fold = np.array(inp['projb'], f32) + bf_ @ projw
    wmap['rw1'] = np.ascontiguousarray(np.array(inp['rw1'], f32))
    wmap['rw2'] = np.ascontiguousarray(np.array(inp['rw2'], f32))
    rw3 = np.zeros((128, 8), f32)
    rw3[:, :6] = np.array(inp['rw3'], f32)
    wmap['rw3'] = rw3
    wmap['tw1'] = np.ascontiguousarray(np.array(inp['tw1'], f32))
    wmap['tw2'] = np.ascontiguousarray(np.array(inp['tw2'], f32))
    tw3 = np.zeros((128, 8), f32)
    tw3[:, :3] = np.array(inp['tw3'], f32)
    wmap['tw3'] = tw3
    hcols = np.zeros((128, 10), f32)
    hcols[:, 0:4] = projb_fold.reshape(4, 128).T
    hcols[:, 4:6] = np.array(inp['rb1'], f32).reshape(2, 128).T
    hcols[:, 6:8] = np.array(inp['tb1'], f32).reshape(2, 128).T
    hcols[:, 8] = np.array(inp['rb2'], f32)
    hcols[:, 9] = np.array(inp['tb2'], f32)
    wmap['hcols'] = hcols
    sb3 = np.zeros((8, 2), f32)
    sb3[0:6, 0] = np.array(inp['rb3'], f32)
    sb3[0:3, 1] = np.array(inp['tb3'], f32)
    wmap['sb3'] = sb3
    wmap['ones_c'] = np.ones((128, 128), f32)
    return wmap


def kernel(**inputs):
    inp = {k: np.asarray(v) for k, v in inputs.items()}

    idx = inp['seed_idxs'].reshape(B, -1).astype(np.int64)      # (B, N)
    sel_seed = np.take_along_axis(np.asarray(inp['fp2_features'], np.float32),
                                  idx[:, None, :], axis=2)
    sel_grasp = np.take_along_axis(np.asarray(inp['local_grasp_features'], np.float32),
                                   idx[:, None, :], axis=2)
    sel_color = np.take_along_axis(np.asarray(inp['local_color_features'], np.float32),
                                   idx[:, None, :], axis=2)
    sel_pose = np.take_along_axis(np.asarray(inp['grasp_pose_feature'], np.float32),
                                  idx[:, None, :], axis=2)
    gsf = np.asarray(inp['sa4_features'], np.float32).max(axis=-1)
    gsf = np.broadcast_to(gsf[:, :, None], (B, 256, NPTS))
    fused = sel_pose + np.concatenate([sel_grasp, sel_color, sel_seed, gsf], axis=1)
    gf = fused.reshape(BE, FRAME, 1024, NPTS)
    cond = np.broadcast_to(gf[:, :1], (BE, NF, 1024, NPTS))
    X = np.concatenate([cond, gf[:, 1:]], axis=2)               # (e, f, 2048, N)

    if 'nc' not in _CACHE:
        _CACHE['nc'] = build_kernel()
    nc = _CACHE['nc']
    wmap = _prep_weights(inp)

    in_maps = []
    for k in range(NCORES):
        xc = X[:, :, :, k * NPC:(k + 1) * NPC]                  # (e, f, c, n)
        xc = xc.transpose(2, 1, 0, 3).reshape(2048, TOK)        # (c, (f,e,n))
        m = dict(wmap)
        m['xin'] = np.ascontiguousarray(xc, dtype=np.float16)
        in_maps.append(m)

    res = run_bass_kernel_spmd(nc, in_maps, core_ids=list(range(NCORES)))
    global _LAST_RES
    _LAST_RES = res

    out = np.zeros((BE * NPTS, NF, 12), np.float32)
    for k in range(NCORES):
        d6 = res.results[k]['d6'][:6]
        tr = res.results[k]['tr3'][:3]
        d6 = d6.reshape(6, NF, BE, NPC).transpose(2, 3, 1, 0)   # (e, n, f, 6)
        tr = tr.reshape(3, NF, BE, NPC).transpose(2, 3, 1, 0)
        rot = _rot6d_np(d6).reshape(BE, NPC, NF, 9)
        for e in range(BE):
            rows = slice(e * NPTS + k * NPC, e * NPTS + (k + 1) * NPC)
            out[rows, :, 0:3] = tr[e]
            out[rows, :, 3:12] = rot[e]
    return out


if __name__ == "__main__":
    build_kernel()
    print("built ok")



# revision 25
# speedup vs baseline: 5.7018x; 5.7018x over previous
import math
import os
import numpy as np
import ml_dtypes
import contextlib

import concourse.bass as bass
import concourse.tile as tile
from concourse import bacc, mybir, masks
from concourse.bass_utils import run_bass_kernel_spmd

F32 = mybir.dt.float32
F32R = mybir.dt.float32r
BF16 = mybir.dt.bfloat16
F16 = mybir.dt.float16
ALU = mybir.AluOpType
AF = mybir.ActivationFunctionType
AX = mybir.AxisListType

NCORES = 8
FRAME = 5
NF = FRAME - 1
D = 512
DH = 64
B = 20
NPTS = 1024
BE = B // FRAME
NPC = NPTS // NCORES     # 128 points per core
TOK = NF * BE * NPC      # 2048 tokens per core
NLAYER = 12
LNEPS = 1e-5
BNEPS = 1e-5
NBN = 16 * NPTS

CONV_DIMS = [2048, 1536, 1024, 768, 512]


def _pe_table(max_len=16, d=D):
    pos = np.arange(max_len, dtype=np.float32)[:, None]
    div = np.exp(np.arange(0, d, 2, dtype=np.float32) * (-math.log(10000.0) / d))
    pe = np.zeros((max_len, d), np.float32)
    pe[:, 0::2] = np.sin(pos * div)
    pe[:, 1::2] = np.cos(pos * div)
    return pe


def build_kernel():
    nc = bacc.Bacc("TRN2", target_bir_lowering=False, debug=False,
                   num_devices=NCORES)

    xin = nc.dram_tensor("xin", [CONV_DIMS[0], TOK], F16, kind="ExternalInput").ap()
    convw = [nc.dram_tensor(f"convw{i}", [CONV_DIMS[i], CONV_DIMS[i + 1]], F16,
                            kind="ExternalInput").ap() for i in range(4)]
    bnconst = [nc.dram_tensor(f"bnconst{i}", [128, 3 * (CONV_DIMS[i + 1] // 128)],
                              F32, kind="ExternalInput").ap() for i in range(3)]
    bias4 = nc.dram_tensor("bias4", [128, 4 * NF], F32, kind="ExternalInput").ap()

    wqkv_d = nc.dram_tensor("wqkv", [NLAYER, D, 3 * D], F32R, kind="ExternalInput").ap()
    # row 0: column-sums of folded Wq|k|v; row 1: folded qkv bias
    wsbq_d = nc.dram_tensor("wsbq", [NLAYER, 2, 3 * D], F32R, kind="ExternalInput").ap()
    wo_d = nc.dram_tensor("wo", [NLAYER, D, D], BF16, kind="ExternalInput").ap()
    w1_d = nc.dram_tensor("w1", [NLAYER, D, D], F32R, kind="ExternalInput").ap()
    w2_d = nc.dram_tensor("w2", [NLAYER, D, D], F32R, kind="ExternalInput").ap()
    tcols_d = nc.dram_tensor("tcols", [NLAYER, 128, 12], F32, kind="ExternalInput").ap()

    projw_d = nc.dram_tensor("projw", [D, D], F32R, kind="ExternalInput").ap()
    rw1_d = nc.dram_tensor("rw1", [D, 256], F32R, kind="ExternalInput").ap()
    rw2_d = nc.dram_tensor("rw2", [256, 128], F32R, kind="ExternalInput").ap()
    rw3_d = nc.dram_tensor("rw3", [128, 8], F32R, kind="ExternalInput").ap()
    tw1_d = nc.dram_tensor("tw1", [D, 256], F32R, kind="ExternalInput").ap()
    tw2_d = nc.dram_tensor("tw2", [256, 128], F32R, kind="ExternalInput").ap()
    tw3_d = nc.dram_tensor("tw3", [128, 8], F32R, kind="ExternalInput").ap()
    hcols_d = nc.dram_tensor("hcols", [128, 10], F32, kind="ExternalInput").ap()
    sb3_d = nc.dram_tensor("sb3", [8, 2], F32, kind="ExternalInput").ap()
    ones_d = nc.dram_tensor("ones_c", [128, 128], F32, kind="ExternalInput").ap()

    d6_o = nc.dram_tensor("d6", [8, TOK], F32, kind="ExternalOutput").ap()
    tr_o = nc.dram_tensor("tr3", [8, TOK], F32, kind="ExternalOutput").ap()
    xdbg_o = None
    if os.environ.get("KHEADS", "1") == "0":
        xdbg_o = nc.dram_tensor("xdbg", [8, 128, TOK], F32,
                                kind="ExternalOutput").ap()

    with tile.TileContext(nc) as tc, contextlib.ExitStack() as ctx:
        const_p = ctx.enter_context(tc.tile_pool(name="consts", bufs=1))
        onescol = const_p.tile([128, 1], F32R)
        onesrow = const_p.tile([1, 128], F32R)
        ident = const_p.tile([128, 128], BF16)
        nc.gpsimd.dma_start(onescol[:], ones_d[:, 0:1])
        nc.gpsimd.dma_start(onesrow[:], ones_d[0:1, :])
        masks.make_identity(nc, ident[:])

        xs_p = ctx.enter_context(tc.tile_pool(name="xstate", bufs=1))
        xA = [xs_p.tile([128, TOK], F32R, tag=f"xA{m}", name=f"xA{m}") for m in range(4)]
        xB = [xs_p.tile([128, TOK], F32R, tag=f"xB{m}", name=f"xB{m}") for m in range(4)]

        rows_p = ctx.enter_context(tc.tile_pool(name="rows", bufs=1))
        stat_p = ctx.enter_context(tc.tile_pool(name="stats", bufs=1))
        scr = ctx.enter_context(tc.tile_pool(name="scratch", bufs=1))
        dram_p = ctx.enter_context(tc.tile_pool(name="dramb", bufs=1, space="DRAM"))

        pp_mm = ctx.enter_context(tc.tile_pool(name="ppmm", bufs=4, space="PSUM"))
        pp_row = ctx.enter_context(tc.tile_pool(name="pprow", bufs=1, space="PSUM"))
        pp_bc = ctx.enter_context(tc.tile_pool(name="ppbc", bufs=2, space="PSUM"))

        y_dram = [dram_p.tile([CONV_DIMS[i], TOK], F16, tag=f"ydram{i}", name=f"ydram{i}")
                  for i in range(1, 4)]

        # ------------------------------------------------------------------
        # conv stack (activations spilled to DRAM, BN applied on load)
        # ------------------------------------------------------------------
        b4sb = stat_p.tile([128, 4 * NF], F32, tag="b4")
        nc.sync.dma_start(b4sb[:], bias4[:])

        bn_s = {}
        bn_t = {}

        def conv_layer(li, wcp, cxp, pp_conv):
            kdim, mdim = CONV_DIMS[li - 1], CONV_DIMS[li]
            KC, MC = kdim // 128, mdim // 128
            src = xin if li == 1 else y_dram[li - 2]
            with_bn = li < 4
            if with_bn:
                sum_acc = stat_p.tile([128, MC * 4], F32, tag=f"sum{li}")
                sq_acc = stat_p.tile([128, MC * 4], F32, tag=f"sq{li}")
            # whole layer's weights resident, loaded once (not per pt chunk);
            # single shared allocation reused across conv layers
            wall = wcp.tile([128, CONV_DIMS[0] // 128 * 12 * 128], F16,
                            tag="wall", name=f"wall{li}")
            for m in range(MC):
                nc.sync.dma_start(
                    wall[:, m * KC * 128:(m + 1) * KC * 128]
                    .rearrange("p (k c) -> p k c", k=KC),
                    convw[li - 1][:, m * 128:(m + 1) * 128]
                    .rearrange("(k p) c -> p k c", p=128))
            for pt in range(4):
                xt = cxp.tile([128, KC * 512], F16, tag="convx", name="convx",
                              bufs=2)
                nc.sync.dma_start(
                    xt[:].rearrange("p (k c) -> p k c", k=KC),
                    src[:, pt * 512:(pt + 1) * 512]
                    .rearrange("(k p) c -> p k c", p=128))
                if li > 1:
                    s_p, t_p = bn_s[li - 1], bn_t[li - 1]
                    for k in range(KC):
                        nc.scalar.activation(
                            xt[:, k * 512:(k + 1) * 512],
                            xt[:, k * 512:(k + 1) * 512],
                            AF.Relu, bias=t_p[:, k:k + 1], scale=s_p[:, k:k + 1])
                for m in range(MC):
                    ps = pp_conv.tile([128, 512], F32, tag="mm", name="cps")
                    for k in range(KC):
                        nc.tensor.matmul(
                            ps[:], wall[:, (m * KC + k) * 128:(m * KC + k + 1) * 128],
                            xt[:, k * 512:(k + 1) * 512],
                            start=(k == 0), stop=(k == KC - 1))
                    if with_bn:
                        ot = cxp.tile([128, 512], F16, tag="convot", name="cot",
                                      bufs=3)
                        nc.scalar.activation(
                            ot[:], ps[:], AF.Copy,
                            accum_out=sum_acc[:, m * 4 + pt:m * 4 + pt + 1])
                        sqs = cxp.tile([128, 512], F16, tag="sqscr", name="sqs",
                                       bufs=3)
                        nc.scalar.activation(
                            sqs[:], ps[:], AF.Square,
                            accum_out=sq_acc[:, m * 4 + pt:m * 4 + pt + 1])
                        nc.sync.dma_start(
                            y_dram[li - 1][m * 128:(m + 1) * 128,
                                           pt * 512:(pt + 1) * 512], ot[:])
                    else:
                        nc.scalar.activation(
                            xA[m][:, pt * 512:(pt + 1) * 512], ps[:], AF.Identity,
                            bias=b4sb[:, m * 4 + pt:m * 4 + pt + 1])
            if not with_bn:
                return
            allin = stat_p.tile([128, 2 * MC], F32, tag=f"ain{li}", name="allin")
            nc.vector.tensor_reduce(
                allin[:, 0:MC], sum_acc[:].rearrange("p (m t) -> p m t", m=MC),
                axis=AX.X, op=ALU.add)
            nc.vector.tensor_reduce(
                allin[:, MC:2 * MC], sq_acc[:].rearrange("p (m t) -> p m t", m=MC),
                axis=AX.X, op=ALU.add)
            bin_ = dram_p.tile([128, 2 * MC], F32, tag=f"arin{li}", name="arin")
            bout = dram_p.tile([128, 2 * MC], F32, tag=f"arout{li}", name="arout")
            nc.sync.dma_start(bin_[:], allin[:])
            nc.gpsimd.collective_compute(
                "AllReduce", ALU.add, replica_groups=[list(range(NCORES))],
                ins=[bin_.opt()], outs=[bout.opt()])
            gl = stat_p.tile([128, 2 * MC], F32, tag=f"gl{li}", name="gl")
            nc.sync.dma_start(gl[:], bout[:])
            cst = stat_p.tile([128, 3 * MC], F32, tag=f"cst{li}", name="cst")
            nc.sync.dma_start(cst[:], bnconst[li - 1][:])
            mu = stat_p.tile([128, MC], F32, tag=f"mu{li}", name="bmu")
            var = stat_p.tile([128, MC], F32, tag=f"va{li}", name="bvar")
            s_t = stat_p.tile([128, MC], F32, tag=f"s{li}", name="bs")
            t_t = stat_p.tile([128, MC], F32, tag=f"t{li}", name="bt")
            nc.scalar.mul(mu[:], gl[:, 0:MC], 1.0 / NBN)
            nc.scalar.mul(var[:], gl[:, MC:2 * MC], 1.0 / NBN)
            msq = stat_p.tile([128, MC], F32, tag=f"ms{li}", name="bmsq")
            nc.vector.tensor_mul(msq[:], mu[:], mu[:])
            nc.vector.tensor_tensor(var[:], var[:], msq[:], op=ALU.subtract)
            nc.vector.tensor_scalar(var[:], var[:], BNEPS, None, op0=ALU.add)
            sd = stat_p.tile([128, MC], F32, tag=f"sd{li}", name="bsd")
            nc.scalar.activation(sd[:], var[:], AF.Sqrt)
            rsd = stat_p.tile([128, MC], F32, tag=f"rs{li}", name="brsd")
            nc.vector.reciprocal(rsd[:], sd[:])
            nc.vector.tensor_mul(s_t[:], rsd[:], cst[:, 0:MC])
            nc.vector.tensor_mul(t_t[:], mu[:], s_t[:])
            nc.vector.tensor_tensor(t_t[:], cst[:, MC:2 * MC], t_t[:],
                                    op=ALU.subtract)
            bn_s[li], bn_t[li] = s_t, t_t

        with tc.tile_pool(name="wcp", bufs=1) as wcp, \
             tc.tile_pool(name="cxp", bufs=1) as cxp:
            for li in (1, 2, 3, 4):
                conv_layer(li, wcp, cxp, pp_mm)

        # ------------------------------------------------------------------
        # transformer
        # ------------------------------------------------------------------
        def ln_cols(xt, xview, dst_tiles, dst_cols):
            """LN per token over feature dim (stats + apply on DVE)."""
            ps_s = pp_row.tile([1, 512], F32, tag="row_s", name="ps_s")
            ps_q = pp_row.tile([1, 512], F32, tag="row_q", name="ps_q")
            for k in range(4):
                nc.tensor.matmul(ps_s[:], onescol[:], xview(k),
                                 start=(k == 0), stop=(k == 3))
            for k in range(4):
                sq = scr.tile([128, 512], F32R, tag="lnsq", name="lnsq")
                nc.scalar.square(sq[:], xview(k))
                nc.tensor.matmul(ps_q[:], onescol[:], sq[:],
                                 start=(k == 0), stop=(k == 3))
            mu = rows_p.tile([1, 512], F32R, tag="mu", name="lmu", bufs=2)
            e2 = rows_p.tile([1, 512], F32, tag="e2", name="le2", bufs=2)
            r = rows_p.tile([1, 512], F32R, tag="r", name="lr", bufs=2)
            nc.scalar.mul(mu[:], ps_s[:], 1.0 / D)
            nc.scalar.mul(e2[:], ps_q[:], 1.0 / D)
            with nc.allow_low_precision(reason="f32r row math"):
                nc.vector.tensor_mul(r[:], mu[:], mu[:])
                nc.vector.tensor_tensor(e2[:], e2[:], r[:], op=ALU.subtract)
                nc.vector.tensor_scalar(e2[:], e2[:], LNEPS, None, op0=ALU.add)
                nc.scalar.activation(e2[:], e2[:], AF.Sqrt)
                nc.vector.reciprocal(r[:], e2[:])
            psb_mu = pp_bc.tile([128, 512], F32, tag="bc", name="psbmu")
            psb_r = pp_bc.tile([128, 512], F32, tag="bc", name="psbr")
            nc.tensor.matmul(psb_mu[:], onesrow[:], mu[:], start=True, stop=True)
            nc.tensor.matmul(psb_r[:], onesrow[:], r[:], start=True, stop=True)
            for k in range(4):
                tmp = scr.tile([128, 512], F32, tag="lntmp", name="lntmp")
                nc.vector.tensor_tensor(tmp[:], xview(k), psb_mu[:],
                                        op=ALU.subtract)
                nc.vector.tensor_mul(dst_tiles[k][:, dst_cols], tmp[:], psb_r[:])

        def ln1_rows(x_in, nmsd, rall):
            """Per-frame LN stats; -mu into nmsd row 0, sd into row 1,
            1/sd into row f of rall (PE-transposed to columns afterwards)."""
            for f in range(4):
                sl = slice(f * 512, (f + 1) * 512)
                ps_s = pp_row.tile([1, 512], F32, tag="row_s", name="ps_s")
                ps_q = pp_row.tile([1, 512], F32, tag="row_q", name="ps_q")
                for k in range(4):
                    nc.tensor.matmul(ps_s[:], onescol[:], x_in[k][:, sl],
                                     start=(k == 0), stop=(k == 3))
                for k in range(4):
                    sq = scr.tile([128, 512], F32R, tag="lnsq", name="lnsq")
                    nc.scalar.square(sq[:], x_in[k][:, sl])
                    nc.tensor.matmul(ps_q[:], onescol[:], sq[:],
                                     start=(k == 0), stop=(k == 3))
                e2 = rows_p.tile([1, 512], F32, tag="e2", name="le2", bufs=2)
                nc.scalar.mul(e2[:], ps_q[:], 1.0 / D)
                with nc.allow_low_precision(reason="ln1 rows"):
                    nc.scalar.mul(nmsd[0:1, sl], ps_s[:], -1.0 / D)
                    msq = rows_p.tile([1, 512], F32, tag="rr", name="lms", bufs=2)
                    nc.scalar.activation(msq[:], ps_s[:], AF.Square,
                                         scale=1.0 / D)
                    nc.vector.tensor_tensor(e2[:], e2[:], msq[:], op=ALU.subtract)
                    nc.vector.tensor_scalar(e2[:], e2[:], LNEPS, None, op0=ALU.add)
                    nc.scalar.activation(nmsd[1:2, sl], e2[:], AF.Sqrt)
                    nc.scalar.activation(e2[:], e2[:], AF.Sqrt)
                    nc.vector.reciprocal(rall[f:f + 1, :], e2[:])

        tr_ctx = ctx.enter_context(contextlib.ExitStack())
        wp = tr_ctx.enter_context(tc.tile_pool(name="wp", bufs=1))
        wqp = tr_ctx.enter_context(tc.tile_pool(name="wqp", bufs=1))
        attn_p = tr_ctx.enter_context(tc.tile_pool(name="attn", bufs=2))
        ot_p = tr_ctx.enter_context(tc.tile_pool(name="otp", bufs=1))
        otb = [ot_p.tile([128, TOK], BF16, tag=f"ot{m}", name=f"otb{m}")
               for m in range(4)]

        def transformer_layer(li, x_in, x_mid):
            wq = [wqp.tile([128, 3 * D], F32R, tag=f"wqkv{k}", name=f"wq{k}")
                  for k in range(4)]
            for k in range(4):
                nc.sync.dma_start(wq[k][:], wqkv_d[li, k * 128:(k + 1) * 128, :])
            wsbq = rows_p.tile([2, 3 * D], F32R, tag="wsbq", name="wsbq", bufs=1)
            nc.sync.dma_start(wsbq[:], wsbq_d[li])
            cols = stat_p.tile([128, 12], F32, tag="tcols", name="tcols")
            nc.sync.dma_start(cols[:], tcols_d[li])

            nmsd = attn_p.tile([2, TOK], F32R, tag="nmsd", name="nmsd", bufs=1)
            rall = attn_p.tile([4, 512], F32R, tag="rall", name="rall", bufs=1)
            rcol = attn_p.tile([128, 16], F32R, tag="rcol", name="rcol", bufs=1)
            ln1_rows(x_in, nmsd, rall)
            rview = rcol[:].rearrange("p (f s) -> p f s", f=4)
            for st in range(4):
                pst4 = pp_bc.tile([128, 4], F32R, tag="bc", name="pst4")
                nc.tensor.transpose(
                    pst4[:], rall[0:4, st * 128:(st + 1) * 128],
                    ident[0:4, 0:4])
                nc.scalar.copy(rview[:, :, st], pst4[:])

            for st in range(4):
                qt = attn_p.tile([128, TOK], BF16, tag="qst", name="qt", bufs=1)
                kt = attn_p.tile([128, TOK], BF16, tag="kst", name="kt", bufs=1)
                vt = attn_p.tile([128, TOK], BF16, tag="vst", name="vt", bufs=1)
                qkv_dst = [qt, kt, vt]
                for f in range(NF):
                    c0 = f * 512 + st * 128
                    for ns in range(3):
                        ps = pp_mm.tile([128, 512], F32, tag="mm", name="qps")
                        for k in range(4):
                            nc.tensor.matmul(
                                ps[:], x_in[k][:, c0:c0 + 128],
                                wq[k][:, ns * 512:(ns + 1) * 512],
                                start=(k == 0), stop=False)
                        nc.tensor.matmul(ps[:], nmsd[0:2, c0:c0 + 128],
                                         wsbq[:, ns * 512:(ns + 1) * 512],
                                         start=False, stop=True)
                        nc.scalar.activation(
                            qkv_dst[ns][:, f * 512:(f + 1) * 512], ps[:],
                            AF.Copy, scale=rcol[:, f * 4 + st:f * 4 + st + 1])

                # --- scores: one batched mul + halving tree over d ---------
                # pbig4 layout (i, j, h, d); tree output s4 layout (i, j, h)
                pbig4 = attn_p.tile([128, 4 * TOK], BF16, tag="bigA",
                                    name="pbig4", bufs=1)
                nc.vector.tensor_mul(
                    pbig4[:].rearrange("p (i j e) -> p i j e", i=4, j=4),
                    qt[:].rearrange("p (i e) -> p i e", i=4).unsqueeze(2)
                    .broadcast_to([128, 4, 4, 512]),
                    kt[:].rearrange("p (j e) -> p j e", j=4).unsqueeze(1)
                    .broadcast_to([128, 4, 4, 512]))
                cA = attn_p.tile([128, 2 * TOK], BF16, tag="midA",
                                 name="cA", bufs=1)
                s4 = attn_p.tile([128, 128], BF16, tag="s_sc", name="s4",
                                 bufs=1)
                with nc.allow_low_precision(reason="qk tree (tiny logits)"):
                    lvl = [(pbig4, 64), (cA, 32), (pbig4, 16), (cA, 8),
                           (pbig4, 4), (cA, 2)]
                    for n, (src, w) in enumerate(lvl):
                        dst, _ = lvl[n + 1] if n + 1 < len(lvl) else (s4, 1)
                        h = w // 2
                        sv = src[:, 0:128 * w].rearrange("p (g d) -> p g d",
                                                         g=128)
                        nc.vector.tensor_tensor(
                            dst[:, 0:128 * h].rearrange("p (g d) -> p g d",
                                                        g=128),
                            sv[:, :, 0:h], sv[:, :, h:w], op=ALU.add)
                # softmax over j without max-subtraction (logits bounded)
                # s4 cols = i*32 + j*8 + h -> exp -> A cols = i*32 + h*4 + j
                eexp = attn_p.tile([128, 128], BF16, tag="eexp", name="eexp",
                                   bufs=1)
                nc.scalar.activation(
                    eexp[:].rearrange("p (i h j) -> p i h j", i=4, h=8),
                    s4[:].rearrange("p (i j h) -> p i h j", i=4, j=4), AF.Exp)
                z = attn_p.tile([128, 32], F32, tag="z", name="zt", bufs=1)
                nc.vector.tensor_reduce(
                    z[:].rearrange("p (i h) -> p i h", i=4),
                    eexp[:].rearrange("p (i h j) -> p i h j", i=4, h=8),
                    axis=AX.X, op=ALU.add)
                zr = attn_p.tile([128, 32], F32, tag="zr", name="zr", bufs=1)
                nc.vector.reciprocal(zr[:], z[:])
                a_t = attn_p.tile([128, 128], BF16, tag="a_t", name="a_t",
                                  bufs=1)
                nc.vector.tensor_mul(
                    a_t[:].rearrange("p (i h j) -> p i h j", i=4, h=8),
                    eexp[:].rearrange("p (i h j) -> p i h j", i=4, h=8),
                    zr[:].rearrange("p (i h) -> p i h", i=4).unsqueeze(3)
                    .broadcast_to([128, 4, 8, 4]))
                # replicate attention weights in pairs so the AV mul keeps
                # packed innermost access (DVE 2x mode)
                arep2 = attn_p.tile([128, 256], BF16, tag="arep2",
                                    name="arep2", bufs=1)
                nc.vector.tensor_copy(
                    arep2[:].rearrange("p (i h j t) -> p i h j t", i=4, h=8,
                                       j=4),
                    a_t[:].rearrange("p (i h j) -> p i h j", i=4, h=8)
                    .unsqueeze(4).broadcast_to([128, 4, 8, 4, 2]))
                # --- AV: per-i muls (2x), then pairwise j-adds -------------
                tbig4 = attn_p.tile([128, 4 * TOK], BF16, tag="bigA",
                                    name="tbig4", bufs=1)
                for i in range(4):
                    nc.vector.tensor_mul(
                        tbig4[:, i * TOK:(i + 1) * TOK]
                        .rearrange("p (j h dp t) -> p j h dp t", j=4, h=8,
                                   dp=32),
                        vt[:].rearrange("p (j h dp t) -> p j h dp t", j=4,
                                        h=8, dp=32),
                        arep2[:, i * 64:(i + 1) * 64]
                        .rearrange("p (h j t) -> p j h t", h=8, j=4)
                        .unsqueeze(3).broadcast_to([128, 4, 8, 32, 2]))
                o2 = attn_p.tile([128, 2 * TOK], BF16, tag="midA", name="o2",
                                 bufs=1)
                with nc.allow_low_precision(reason="av pair add"):
                    nc.vector.tensor_tensor(
                        o2[:].rearrange("p (i e) -> p i e", i=4),
                        tbig4[:].rearrange("p (i e) -> p i e", i=4)[:, :, 0:1024],
                        tbig4[:].rearrange("p (i e) -> p i e", i=4)[:, :, 1024:2048],
                        op=ALU.add)
                of32 = attn_p.tile([128, TOK], F32R, tag="oall", name="of32",
                                   bufs=1)
                with nc.allow_low_precision(reason="f32r out"):
                    nc.vector.tensor_tensor(
                        of32[:].rearrange("p (i e) -> p i e", i=4),
                        o2[:].rearrange("p (i e) -> p i e", i=4)[:, :, 0:512],
                        o2[:].rearrange("p (i e) -> p i e", i=4)[:, :, 512:1024],
                        op=ALU.add)
                for i in range(4):
                    for c in range(4):
                        pst = pp_bc.tile([128, 128], F32R, tag="bc", name="pst")
                        nc.tensor.transpose(
                            pst[:], of32[:, i * 512 + c * 128:
                                         i * 512 + (c + 1) * 128], ident[:])
                        nc.scalar.copy(
                            otb[c][:, i * 512 + st * 128:i * 512 + st * 128 + 128],
                            pst[:])

            wo = [wp.tile([128, D], BF16, tag=f"wo{k}", name=f"wo{k}")
                  for k in range(4)]
            for k in range(4):
                nc.sync.dma_start(wo[k][:], wo_d[li, k * 128:(k + 1) * 128, :])
            for m in range(4):
                for ns in range(4):
                    ps = pp_mm.tile([128, 512], F32, tag="mm", name="ops")
                    for k in range(4):
                        nc.tensor.matmul(
                            ps[:], wo[k][:, m * 128:(m + 1) * 128],
                            otb[k][:, ns * 512:(ns + 1) * 512],
                            start=(k == 0), stop=(k == 3))
                    nc.vector.scalar_tensor_tensor(
                        x_mid[m][:, ns * 512:(ns + 1) * 512], ps[:],
                        cols[:, 4 + m:5 + m], x_in[m][:, ns * 512:(ns + 1) * 512],
                        op0=ALU.add, op1=ALU.add)

            w1 = [wp.tile([128, D], F32R, tag=f"w1_{k}", name=f"w1_{k}")
                  for k in range(4)]
            w2 = [wp.tile([128, D], F32R, tag=f"w2_{k}", name=f"w2_{k}")
                  for k in range(4)]
            for k in range(4):
                nc.sync.dma_start(w1[k][:], w1_d[li, k * 128:(k + 1) * 128, :])
                nc.sync.dma_start(w2[k][:], w2_d[li, k * 128:(k + 1) * 128, :])
            for ns in range(4):
                # scratch aliased onto dead attention buffers (same tags)
                xh2_all = attn_p.tile([128, 2048], F32R, tag="oall",
                                      name="xh2_all", bufs=1)
                xh2 = [xh2_all[:, k * 512:(k + 1) * 512] for k in range(4)]
                ln_cols(x_mid,
                        lambda k: x_mid[k][:, ns * 512:(ns + 1) * 512],
                        xh2, slice(0, 512))
                hsl_all = attn_p.tile([128, 2048], F32R, tag="midA",
                                      name="hsl_all", bufs=1)
                hsl = [hsl_all[:, m * 512:(m + 1) * 512] for m in range(4)]
                for m in range(4):
                    ps = pp_mm.tile([128, 512], F32, tag="mm", name="m1ps")
                    for k in range(4):
                        nc.tensor.matmul(
                            ps[:], w1[k][:, m * 128:(m + 1) * 128], xh2[k][:],
                            start=(k == 0), stop=(k == 3))
                    nc.scalar.activation(hsl[m][:], ps[:], AF.Gelu_apprx_tanh,
                                         bias=cols[:, m:m + 1])
                for m in range(4):
                    ps = pp_mm.tile([128, 512], F32, tag="mm", name="m2ps")
                    for k in range(4):
                        nc.tensor.matmul(
                            ps[:], w2[k][:, m * 128:(m + 1) * 128], hsl[k][:],
                            start=(k == 0), stop=(k == 3))
                    nc.vector.scalar_tensor_tensor(
                        x_mid[m][:, ns * 512:(ns + 1) * 512], ps[:],
                        cols[:, 8 + m:9 + m], x_mid[m][:, ns * 512:(ns + 1) * 512],
                        op0=ALU.add, op1=ALU.add)

        cur, alt = xA, xB
        _nl = int(os.environ.get("KNLAYERS", NLAYER))
        _bar = os.environ.get("KBAR", "0") == "1"
        for li in range(_nl):
            transformer_layer(li, cur, alt)
            cur, alt = alt, cur
            if _bar:
                nc.all_engine_barrier()

        tr_ctx.close()

        # ------------------------------------------------------------------
        # heads
        # ------------------------------------------------------------------
        _skip_heads = os.environ.get("KHEADS", "1") == "0"
        if _skip_heads:
            _da, _db = (int(c) for c in os.environ.get("KDUMP", "01"))
            nc.gpsimd.dma_start(d6_o[:], cur[_da][0:8, :])
            nc.gpsimd.dma_start(tr_o[:], cur[_db][0:8, :])
            for m in range(4):
                nc.gpsimd.dma_start(xdbg_o[m], cur[m][:])
                nc.gpsimd.dma_start(xdbg_o[4 + m], alt[m][:])
        with tc.tile_pool(name="heads", bufs=1) as hp, \
             tc.tile_pool(name="whp", bufs=1) as whp:
          if not _skip_heads:
              hc = stat_p.tile([128, 10], F32, tag="hcols", name="hc")
              nc.sync.dma_start(hc[:], hcols_d[:])
              sb3 = stat_p.tile([8, 2], F32, tag="sb3", name="sb3")
              nc.sync.dma_start(sb3[:], sb3_d[:])

              xhf = [hp.tile([128, TOK], F32R, tag=f"xhf{k}", name=f"xhf{k}")
                     for k in range(4)]
              for st in range(4):
                  ln_cols(cur, lambda k: cur[k][:, st * 512:(st + 1) * 512],
                          xhf, slice(st * 512, (st + 1) * 512))

              xp = alt

              def mm_head(src_tiles, wt_dram, kdim, mdim, dst_tiles, evict):
                  KC = kdim // 128
                  MC = max(mdim // 128, 1)
                  wsb = [whp.tile([128, mdim], F32R, tag=f"wh_{kdim}_{mdim}_{k}",
                                  name=f"wh{k}") for k in range(KC)]
                  for k in range(KC):
                      nc.sync.dma_start(wsb[k][:], wt_dram[k * 128:(k + 1) * 128, :])
                  for m in range(MC):
                      for ns in range(4):
                          ps = pp_mm.tile([128, 512], F32, tag="mm", name="hps")
                          for k in range(KC):
                              nc.tensor.matmul(
                                  ps[:], wsb[k][:, m * 128:(m + 1) * 128],
                                  src_tiles[k][:, ns * 512:(ns + 1) * 512],
                                  start=(k == 0), stop=(k == KC - 1))
                          evict(ps, dst_tiles[m], m, ns)

              mm_head(xhf, projw_d, D, D, xp,
                      lambda ps, dst, m, ns: nc.scalar.activation(
                          dst[:, ns * 512:(ns + 1) * 512], ps[:], AF.Identity,
                          bias=hc[:, m:m + 1]))

              def branch(w1d, w2d, w3d, b1ofs, b2ofs, out_dram, b3col, r1, r2, pfx):
                  mm_head(xp, w1d, D, 256, r1,
                          lambda ps, dst, m, ns: nc.scalar.activation(
                              dst[:, ns * 512:(ns + 1) * 512], ps[:], AF.Relu,
                              bias=hc[:, b1ofs + m:b1ofs + m + 1]))
                  mm_head(r1, w2d, 256, 128, r2,
                          lambda ps, dst, m, ns: nc.scalar.activation(
                              dst[:, ns * 512:(ns + 1) * 512], ps[:], AF.Relu,
                              bias=hc[:, b2ofs:b2ofs + 1]))
                  w3 = whp.tile([128, 8], F32R, tag=f"w3{pfx}", name="w3")
                  nc.sync.dma_start(w3[:], w3d[:])
                  out_sb = hp.tile([8, TOK], F32, tag=f"{pfx}out", name=f"{pfx}out")
                  for ns in range(4):
                      ps = pp_mm.tile([8, 512], F32, tag="mm", name="bps")
                      nc.tensor.matmul(ps[:], w3[:], r2[0][:, ns * 512:(ns + 1) * 512],
                                       start=True, stop=True)
                      nc.scalar.activation(out_sb[:, ns * 512:(ns + 1) * 512], ps[:],
                                           AF.Identity, bias=b3col)
                  nc.sync.dma_start(out_dram[:], out_sb[:])

              # reuse dead transformer buffers for intermediates
              branch(rw1_d, rw2_d, rw3_d, 4, 8, d6_o, sb3[:, 0:1],
                     [cur[0], cur[1]], [xhf[0]], "r")
              branch(tw1_d, tw2_d, tw3_d, 6, 9, tr_o, sb3[:, 1:2],
                     [cur[2], cur[3]], [xhf[1]], "t")

    nc.compile()
    return nc


# ----------------------------------------------------------------------------
# host side
# ----------------------------------------------------------------------------

_CACHE = {}


def _normalize_np(v, eps=1e-12):
    return v / np.maximum(np.linalg.norm(v, axis=-1, keepdims=True), eps)


def _rot6d_np(d6):
    a1, a2 = d6[..., :3], d6[..., 3:]
    b1 = _normalize_np(a1)
    b2 = _normalize_np(a2 - np.sum(b1 * a2, -1, keepdims=True) * b1)
    b3 = np.cross(b1, b2)
    return np.stack([b1, b2, b3], axis=-2)


def _prep_weights(inp):
    f32 = np.float32
    wmap = {}
    for i, cw in enumerate(['c1w', 'c2w', 'c3w', 'c4w']):
        wmap[f'convw{i}'] = np.ascontiguousarray(inp[cw].T.astype(np.float16))
    for i, (g, b2) in enumerate([('bn1g', 'bn1b'), ('bn2g', 'bn2b'),
                                 ('bn3g', 'bn3b')]):
        M = CONV_DIMS[i + 1] // 128
        bn = np.concatenate([
            inp[g].reshape(M, 128).T, inp[b2].reshape(M, 128).T,
            inp[f'c{i + 1}b'].reshape(M, 128).T], axis=1)
        wmap[f'bnconst{i}'] = np.ascontiguousarray(bn.astype(f32))
    pe = _pe_table()[:NF]
    b4 = inp['c4b'][None, :].astype(f32) + pe                   # (4, 512)
    # cols: m*4 + pt ; frame index == pt
    wmap['bias4'] = np.ascontiguousarray(
        b4.reshape(NF, 4, 128).transpose(2, 1, 0).reshape(128, 16).astype(f32))

    qkvw = np.array(inp['qkvw'], f32)
    qkvb = np.array(inp['qkvb'], f32)
    qkvw[:, :, :512] /= math.sqrt(DH)
    qkvb[:, :512] /= math.sqrt(DH)
    g1 = np.array(inp['ln1g'], f32)
    b1 = np.array(inp['ln1b'], f32)
    wq_fold = g1[:, :, None] * qkvw
    bq_fold = qkvb + np.einsum('ld,ldf->lf', b1, qkvw)
    wmap['wqkv'] = np.ascontiguousarray(wq_fold.astype(f32))
    wmap['wsbq'] = np.ascontiguousarray(np.stack(
        [wq_fold.sum(axis=1), bq_fold], axis=1).astype(f32))
    wmap['wo'] = np.ascontiguousarray(np.array(inp['outw'], f32)
                                      .astype(ml_dtypes.bfloat16))
    g2 = np.array(inp['ln2g'], f32)
    bl2 = np.array(inp['ln2b'], f32)
    m1w = np.array(inp['m1w'], f32)
    w1_fold = g2[:, :, None] * m1w
    b1_fold = np.array(inp['m1b'], f32) + np.einsum('ld,ldf->lf', bl2, m1w)
    wmap['w1'] = np.ascontiguousarray(w1_fold.astype(f32))
    wmap['w2'] = np.ascontiguousarray(np.array(inp['m2w'], f32))
    cols = np.zeros((NLAYER, 128, 12), f32)
    cols[:, :, 0:4] = b1_fold.reshape(NLAYER, 4, 128).transpose(0, 2, 1)
    cols[:, :, 4:8] = np.array(inp['outb'], f32).reshape(NLAYER, 4, 128) \
        .transpose(0, 2, 1)
    cols[:, :, 8:12] = np.array(inp['m2b'], f32).reshape(NLAYER, 4, 128) \
        .transpose(0, 2, 1)
    wmap['tcols'] = cols

    gf_ = np.array(inp['lnfg'], f32)
    bf_ = np.array(inp['lnfb'], f32)
    projw = np.array(inp['projw'], f32)
    wmap['projw'] = np.ascontiguousarray(gf_[:, None] * projw)
    projb_fold = np.array(inp['projb'], f32) + bf_ @ projw
    wmap['rw1'] = np.ascontiguousarray(np.array(inp['rw1'], f32))
    wmap['rw2'] = np.ascontiguousarray(np.array(inp['rw2'], f32))
    rw3 = np.zeros((128, 8), f32)
    rw3[:, :6] = np.array(inp['rw3'], f32)
    wmap['rw3'] = rw3
    wmap['tw1'] = np.ascontiguousarray(np.array(inp['tw1'], f32))
    wmap['tw2'] = np.ascontiguousarray(np.array(inp['tw2'], f32))
    tw3 = np.zeros((128, 8), f32)
    tw3[:, :3] = np.array(inp['tw3'], f32)
    wmap['tw3'] = tw3
    hcols = np.zeros((128, 10), f32)
    hcols[:, 0:4] = projb_fold.reshape(4, 128).T
    hcols[:, 4:6] = np.array(inp['rb1'], f32).reshape(2, 128).T
    hcols[:, 6:8] = np.array(inp['tb1'], f32).reshape(2, 128).T
    hcols[:, 8] = np.array(inp['rb2'], f32)
    hcols[:, 9] = np.array(inp['tb2'], f32)
    wmap['hcols'] = hcols
    sb3 = np.zeros((8, 2), f32)
    sb3[0:6, 0] = np.array(inp['rb3'], f32)
    sb3[0:3, 1] = np.array(inp['tb3'], f32)
    wmap['sb3'] = sb3
    wmap['ones_c'] = np.ones((128, 128), f32)
    return wmap


def kernel(**inputs):
    inp = {k: np.asarray(v) for k, v in inputs.items()}

    idx = inp['seed_idxs'].reshape(B, -1).astype(np.int64)      # (B, N)
    sel_seed = np.take_along_axis(np.asarray(inp['fp2_features'], np.float32),
                                  idx[:, None, :], axis=2)
    sel_grasp = np.take_along_axis(np.asarray(inp['local_grasp_features'], np.float32),
                                   idx[:, None, :], axis=2)
    sel_color = np.take_along_axis(np.asarray(inp['local_color_features'], np.float32),
                                   idx[:, None, :], axis=2)
    sel_pose = np.take_along_axis(np.asarray(inp['grasp_pose_feature'], np.float32),
                                  idx[:, None, :], axis=2)
    gsf = np.asarray(inp['sa4_features'], np.float32).max(axis=-1)
    gsf = np.broadcast_to(gsf[:, :, None], (B, 256, NPTS))
    fused = sel_pose + np.concatenate([sel_grasp, sel_color, sel_seed, gsf], axis=1)
    gf = fused.reshape(BE, FRAME, 1024, NPTS)
    cond = np.broadcast_to(gf[:, :1], (BE, NF, 1024, NPTS))
    X = np.concatenate([cond, gf[:, 1:]], axis=2)               # (e, f, 2048, N)

    if 'nc' not in _CACHE:
        _CACHE['nc'] = build_kernel()
    nc = _CACHE['nc']
    wmap = _prep_weights(inp)

    in_maps = []
    for k in range(NCORES):
        xc = X[:, :, :, k * NPC:(k + 1) * NPC]                  # (e, f, c, n)
        xc = xc.transpose(2, 1, 0, 3).reshape(2048, TOK)        # (c, (f,e,n))
        m = dict(wmap)
        m['xin'] = np.ascontiguousarray(xc, dtype=np.float16)
        in_maps.append(m)

    res = run_bass_kernel_spmd(nc, in_maps, core_ids=list(range(NCORES)))
    global _LAST_RES
    _LAST_RES = res

    out = np.zeros((BE * NPTS, NF, 12), np.float32)
    for k in range(NCORES):
        d6 = res.results[k]['d6'][:6]
        tr = res.results[k]['tr3'][:3]
        d6 = d6.reshape(6, NF, BE, NPC).transpose(2, 3, 1, 0)   # (e, n, f, 6)
        tr = tr.reshape(3, NF, BE, NPC).transpose(2, 3, 1, 0)
        rot = _rot6d_np(d6).reshape(BE, NPC, NF, 9)
        for e in range(BE):
            rows = slice(e * NPTS + k * NPC, e * NPTS + (k + 1) * NPC)
            out[rows, :, 0:3] = tr[e]
            out[rows, :, 3:12] = rot[e]
    return out


if __name__ == "__main__":
    build_kernel()
    print("built ok")



# revision 27
# speedup vs baseline: 5.7052x; 1.0006x over previous
import math
import os
import numpy as np
import ml_dtypes
import contextlib

import concourse.bass as bass
import concourse.tile as tile
from concourse import bacc, mybir, masks
from concourse.bass_utils import run_bass_kernel_spmd

F32 = mybir.dt.float32
F32R = mybir.dt.float32r
BF16 = mybir.dt.bfloat16
F16 = mybir.dt.float16
ALU = mybir.AluOpType
AF = mybir.ActivationFunctionType
AX = mybir.AxisListType

NCORES = 8
FRAME = 5
NF = FRAME - 1
D = 512
DH = 64
B = 20
NPTS = 1024
BE = B // FRAME
NPC = NPTS // NCORES     # 128 points per core
TOK = NF * BE * NPC      # 2048 tokens per core
NLAYER = 12
LNEPS = 1e-5
BNEPS = 1e-5
NBN = 16 * NPTS

CONV_DIMS = [2048, 1536, 1024, 768, 512]


def _pe_table(max_len=16, d=D):
    pos = np.arange(max_len, dtype=np.float32)[:, None]
    div = np.exp(np.arange(0, d, 2, dtype=np.float32) * (-math.log(10000.0) / d))
    pe = np.zeros((max_len, d), np.float32)
    pe[:, 0::2] = np.sin(pos * div)
    pe[:, 1::2] = np.cos(pos * div)
    return pe


def build_kernel():
    nc = bacc.Bacc("TRN2", target_bir_lowering=False, debug=False,
                   num_devices=NCORES)

    xin = nc.dram_tensor("xin", [CONV_DIMS[0], TOK], F16, kind="ExternalInput").ap()
    convw = [nc.dram_tensor(f"convw{i}", [CONV_DIMS[i], CONV_DIMS[i + 1]], F16,
                            kind="ExternalInput").ap() for i in range(4)]
    bnconst = [nc.dram_tensor(f"bnconst{i}", [128, 3 * (CONV_DIMS[i + 1] // 128)],
                              F32, kind="ExternalInput").ap() for i in range(3)]
    bias4 = nc.dram_tensor("bias4", [128, 4 * NF], F32, kind="ExternalInput").ap()

    wqkv_d = nc.dram_tensor("wqkv", [NLAYER, D, 3 * D], F32R, kind="ExternalInput").ap()
    # row 0: column-sums of folded Wq|k|v; row 1: folded qkv bias
    wsbq_d = nc.dram_tensor("wsbq", [NLAYER, 2, 3 * D], F32R, kind="ExternalInput").ap()
    wo_d = nc.dram_tensor("wo", [NLAYER, D, D], BF16, kind="ExternalInput").ap()
    w1_d = nc.dram_tensor("w1", [NLAYER, D, D], F32R, kind="ExternalInput").ap()
    w2_d = nc.dram_tensor("w2", [NLAYER, D, D], F32R, kind="ExternalInput").ap()
    tcols_d = nc.dram_tensor("tcols", [NLAYER, 128, 12], F32, kind="ExternalInput").ap()

    projw_d = nc.dram_tensor("projw", [D, D], F32R, kind="ExternalInput").ap()
    rw1_d = nc.dram_tensor("rw1", [D, 256], F32R, kind="ExternalInput").ap()
    rw2_d = nc.dram_tensor("rw2", [256, 128], F32R, kind="ExternalInput").ap()
    rw3_d = nc.dram_tensor("rw3", [128, 8], F32R, kind="ExternalInput").ap()
    tw1_d = nc.dram_tensor("tw1", [D, 256], F32R, kind="ExternalInput").ap()
    tw2_d = nc.dram_tensor("tw2", [256, 128], F32R, kind="ExternalInput").ap()
    tw3_d = nc.dram_tensor("tw3", [128, 8], F32R, kind="ExternalInput").ap()
    hcols_d = nc.dram_tensor("hcols", [128, 10], F32, kind="ExternalInput").ap()
    sb3_d = nc.dram_tensor("sb3", [8, 2], F32, kind="ExternalInput").ap()
    ones_d = nc.dram_tensor("ones_c", [128, 128], F32, kind="ExternalInput").ap()

    d6_o = nc.dram_tensor("d6", [8, TOK], F32, kind="ExternalOutput").ap()
    tr_o = nc.dram_tensor("tr3", [8, TOK], F32, kind="ExternalOutput").ap()
    xdbg_o = None
    if os.environ.get("KHEADS", "1") == "0":
        xdbg_o = nc.dram_tensor("xdbg", [8, 128, TOK], F32,
                                kind="ExternalOutput").ap()

    with tile.TileContext(nc) as tc, contextlib.ExitStack() as ctx:
        const_p = ctx.enter_context(tc.tile_pool(name="consts", bufs=1))
        onescol = const_p.tile([128, 1], F32R)
        onesrow = const_p.tile([1, 128], F32R)
        onescol16 = const_p.tile([128, 1], F16)
        onesrow16 = const_p.tile([1, 128], F16)
        ident = const_p.tile([128, 128], BF16)
        nc.gpsimd.dma_start(onescol[:], ones_d[:, 0:1])
        nc.gpsimd.dma_start(onesrow[:], ones_d[0:1, :])
        nc.scalar.copy(onescol16[:], onescol[:])
        nc.scalar.copy(onesrow16[:], onesrow[:])
        masks.make_identity(nc, ident[:])

        xs_p = ctx.enter_context(tc.tile_pool(name="xstate", bufs=1))
        xA = [xs_p.tile([128, TOK], F32R, tag=f"xA{m}", name=f"xA{m}") for m in range(4)]
        xB = [xs_p.tile([128, TOK], F32R, tag=f"xB{m}", name=f"xB{m}") for m in range(4)]

        rows_p = ctx.enter_context(tc.tile_pool(name="rows", bufs=1))
        stat_p = ctx.enter_context(tc.tile_pool(name="stats", bufs=1))
        scr = ctx.enter_context(tc.tile_pool(name="scratch", bufs=1))
        dram_p = ctx.enter_context(tc.tile_pool(name="dramb", bufs=1, space="DRAM"))

        pp_mm = ctx.enter_context(tc.tile_pool(name="ppmm", bufs=4, space="PSUM"))
        pp_row = ctx.enter_context(tc.tile_pool(name="pprow", bufs=1, space="PSUM"))
        pp_bc = ctx.enter_context(tc.tile_pool(name="ppbc", bufs=2, space="PSUM"))

        y_dram = [dram_p.tile([CONV_DIMS[i], TOK], F16, tag=f"ydram{i}", name=f"ydram{i}")
                  for i in range(1, 4)]

        # ------------------------------------------------------------------
        # conv stack (activations spilled to DRAM, BN applied on load)
        # ------------------------------------------------------------------
        b4sb = stat_p.tile([128, 4 * NF], F32, tag="b4")
        nc.sync.dma_start(b4sb[:], bias4[:])

        bn_s = {}
        bn_t = {}

        def conv_layer(li, wcp, cxp, pp_conv):
            kdim, mdim = CONV_DIMS[li - 1], CONV_DIMS[li]
            KC, MC = kdim // 128, mdim // 128
            src = xin if li == 1 else y_dram[li - 2]
            with_bn = li < 4
            if with_bn:
                sum_acc = stat_p.tile([128, MC * 4], F32, tag=f"sum{li}")
                sq_acc = stat_p.tile([128, MC * 4], F32, tag=f"sq{li}")
            # whole layer's weights resident, loaded once (not per pt chunk);
            # single shared allocation reused across conv layers
            wall = wcp.tile([128, CONV_DIMS[0] // 128 * 12 * 128], F16,
                            tag="wall", name=f"wall{li}")
            for m in range(MC):
                nc.sync.dma_start(
                    wall[:, m * KC * 128:(m + 1) * KC * 128]
                    .rearrange("p (k c) -> p k c", k=KC),
                    convw[li - 1][:, m * 128:(m + 1) * 128]
                    .rearrange("(k p) c -> p k c", p=128))
            for pt in range(4):
                xt = cxp.tile([128, KC * 512], F16, tag="convx", name="convx",
                              bufs=2)
                nc.sync.dma_start(
                    xt[:].rearrange("p (k c) -> p k c", k=KC),
                    src[:, pt * 512:(pt + 1) * 512]
                    .rearrange("(k p) c -> p k c", p=128))
                if li > 1:
                    s_p, t_p = bn_s[li - 1], bn_t[li - 1]
                    for k in range(KC):
                        nc.scalar.activation(
                            xt[:, k * 512:(k + 1) * 512],
                            xt[:, k * 512:(k + 1) * 512],
                            AF.Relu, bias=t_p[:, k:k + 1], scale=s_p[:, k:k + 1])
                for m in range(MC):
                    ps = pp_conv.tile([128, 512], F32, tag="mm", name="cps")
                    for k in range(KC):
                        nc.tensor.matmul(
                            ps[:], wall[:, (m * KC + k) * 128:(m * KC + k + 1) * 128],
                            xt[:, k * 512:(k + 1) * 512],
                            start=(k == 0), stop=(k == KC - 1))
                    if with_bn:
                        ot = cxp.tile([128, 512], F16, tag="convot", name="cot",
                                      bufs=3)
                        nc.scalar.activation(
                            ot[:], ps[:], AF.Copy,
                            accum_out=sum_acc[:, m * 4 + pt:m * 4 + pt + 1])
                        sqs = cxp.tile([128, 512], F16, tag="sqscr", name="sqs",
                                       bufs=3)
                        nc.scalar.activation(
                            sqs[:], ps[:], AF.Square,
                            accum_out=sq_acc[:, m * 4 + pt:m * 4 + pt + 1])
                        nc.sync.dma_start(
                            y_dram[li - 1][m * 128:(m + 1) * 128,
                                           pt * 512:(pt + 1) * 512], ot[:])
                    else:
                        nc.scalar.activation(
                            xA[m][:, pt * 512:(pt + 1) * 512], ps[:], AF.Identity,
                            bias=b4sb[:, m * 4 + pt:m * 4 + pt + 1])
            if not with_bn:
                return
            allin = stat_p.tile([128, 2 * MC], F32, tag=f"ain{li}", name="allin")
            nc.vector.tensor_reduce(
                allin[:, 0:MC], sum_acc[:].rearrange("p (m t) -> p m t", m=MC),
                axis=AX.X, op=ALU.add)
            nc.vector.tensor_reduce(
                allin[:, MC:2 * MC], sq_acc[:].rearrange("p (m t) -> p m t", m=MC),
                axis=AX.X, op=ALU.add)
            bin_ = dram_p.tile([128, 2 * MC], F32, tag=f"arin{li}", name="arin")
            bout = dram_p.tile([128, 2 * MC], F32, tag=f"arout{li}", name="arout")
            nc.sync.dma_start(bin_[:], allin[:])
            nc.gpsimd.collective_compute(
                "AllReduce", ALU.add, replica_groups=[list(range(NCORES))],
                ins=[bin_.opt()], outs=[bout.opt()])
            gl = stat_p.tile([128, 2 * MC], F32, tag=f"gl{li}", name="gl")
            nc.sync.dma_start(gl[:], bout[:])
            cst = stat_p.tile([128, 3 * MC], F32, tag=f"cst{li}", name="cst")
            nc.sync.dma_start(cst[:], bnconst[li - 1][:])
            mu = stat_p.tile([128, MC], F32, tag=f"mu{li}", name="bmu")
            var = stat_p.tile([128, MC], F32, tag=f"va{li}", name="bvar")
            s_t = stat_p.tile([128, MC], F32, tag=f"s{li}", name="bs")
            t_t = stat_p.tile([128, MC], F32, tag=f"t{li}", name="bt")
            nc.scalar.mul(mu[:], gl[:, 0:MC], 1.0 / NBN)
            nc.scalar.mul(var[:], gl[:, MC:2 * MC], 1.0 / NBN)
            msq = stat_p.tile([128, MC], F32, tag=f"ms{li}", name="bmsq")
            nc.vector.tensor_mul(msq[:], mu[:], mu[:])
            nc.vector.tensor_tensor(var[:], var[:], msq[:], op=ALU.subtract)
            nc.vector.tensor_scalar(var[:], var[:], BNEPS, None, op0=ALU.add)
            sd = stat_p.tile([128, MC], F32, tag=f"sd{li}", name="bsd")
            nc.scalar.activation(sd[:], var[:], AF.Sqrt)
            rsd = stat_p.tile([128, MC], F32, tag=f"rs{li}", name="brsd")
            nc.vector.reciprocal(rsd[:], sd[:])
            nc.vector.tensor_mul(s_t[:], rsd[:], cst[:, 0:MC])
            nc.vector.tensor_mul(t_t[:], mu[:], s_t[:])
            nc.vector.tensor_tensor(t_t[:], cst[:, MC:2 * MC], t_t[:],
                                    op=ALU.subtract)
            bn_s[li], bn_t[li] = s_t, t_t

        with tc.tile_pool(name="wcp", bufs=1) as wcp, \
             tc.tile_pool(name="cxp", bufs=1) as cxp:
            for li in (1, 2, 3, 4):
                conv_layer(li, wcp, cxp, pp_mm)

        # ------------------------------------------------------------------
        # transformer
        # ------------------------------------------------------------------
        def ln_cols(xt, xview, dst_tiles, dst_cols):
            """LN per token over feature dim (stats + apply on DVE)."""
            ps_s = pp_row.tile([1, 512], F32, tag="row_s", name="ps_s")
            ps_q = pp_row.tile([1, 512], F32, tag="row_q", name="ps_q")
            for k in range(4):
                nc.tensor.matmul(ps_s[:], onescol[:], xview(k),
                                 start=(k == 0), stop=(k == 3))
            for k in range(4):
                sq = scr.tile([128, 512], F16, tag="lnsq", name="lnsq")
                nc.scalar.square(sq[:], xview(k))
                nc.tensor.matmul(ps_q[:], onescol16[:], sq[:],
                                 start=(k == 0), stop=(k == 3))
            mu = rows_p.tile([1, 512], F16, tag="mu", name="lmu", bufs=2)
            e2 = rows_p.tile([1, 512], F32, tag="e2", name="le2", bufs=2)
            r = rows_p.tile([1, 512], F16, tag="r", name="lr", bufs=2)
            nc.scalar.mul(mu[:], ps_s[:], 1.0 / D)
            nc.scalar.mul(e2[:], ps_q[:], 1.0 / D)
            with nc.allow_low_precision(reason="f32r row math"):
                nc.vector.tensor_mul(r[:], mu[:], mu[:])
                nc.vector.tensor_tensor(e2[:], e2[:], r[:], op=ALU.subtract)
                nc.vector.tensor_scalar(e2[:], e2[:], LNEPS, None, op0=ALU.add)
                nc.scalar.activation(e2[:], e2[:], AF.Sqrt)
                nc.vector.reciprocal(r[:], e2[:])
            psb_mu = pp_bc.tile([128, 512], F32, tag="bc", name="psbmu")
            psb_r = pp_bc.tile([128, 512], F32, tag="bc", name="psbr")
            nc.tensor.matmul(psb_mu[:], onesrow16[:], mu[:], start=True, stop=True)
            nc.tensor.matmul(psb_r[:], onesrow16[:], r[:], start=True, stop=True)
            for k in range(4):
                tmp = scr.tile([128, 512], F16, tag="lntmp", name="lntmp")
                nc.vector.tensor_tensor(tmp[:], xview(k), psb_mu[:],
                                        op=ALU.subtract)
                nc.vector.tensor_mul(dst_tiles[k][:, dst_cols], tmp[:], psb_r[:])

        def ln1_rows(x_in, nmsd, rall):
            """Per-frame LN stats; -mu into nmsd row 0, sd into row 1,
            1/sd into row f of rall (PE-transposed to columns afterwards)."""
            for f in range(4):
                sl = slice(f * 512, (f + 1) * 512)
                ps_s = pp_row.tile([1, 512], F32, tag="row_s", name="ps_s")
                ps_q = pp_row.tile([1, 512], F32, tag="row_q", name="ps_q")
                for k in range(4):
                    nc.tensor.matmul(ps_s[:], onescol[:], x_in[k][:, sl],
                                     start=(k == 0), stop=(k == 3))
                for k in range(4):
                    sq = scr.tile([128, 512], F16, tag="lnsq", name="lnsq")
                    nc.scalar.square(sq[:], x_in[k][:, sl])
                    nc.tensor.matmul(ps_q[:], onescol16[:], sq[:],
                                     start=(k == 0), stop=(k == 3))
                e2 = rows_p.tile([1, 512], F32, tag="e2", name="le2", bufs=2)
                nc.scalar.mul(e2[:], ps_q[:], 1.0 / D)
                with nc.allow_low_precision(reason="ln1 rows"):
                    nc.scalar.mul(nmsd[0:1, sl], ps_s[:], -1.0 / D)
                    msq = rows_p.tile([1, 512], F32, tag="rr", name="lms", bufs=2)
                    nc.scalar.activation(msq[:], ps_s[:], AF.Square,
                                         scale=1.0 / D)
                    nc.vector.tensor_tensor(e2[:], e2[:], msq[:], op=ALU.subtract)
                    nc.vector.tensor_scalar(e2[:], e2[:], LNEPS, None, op0=ALU.add)
                    nc.scalar.activation(nmsd[1:2, sl], e2[:], AF.Sqrt)
                    nc.scalar.activation(e2[:], e2[:], AF.Sqrt)
                    nc.vector.reciprocal(rall[f:f + 1, :], e2[:])

        tr_ctx = ctx.enter_context(contextlib.ExitStack())
        wp = tr_ctx.enter_context(tc.tile_pool(name="wp", bufs=1))
        wqp = tr_ctx.enter_context(tc.tile_pool(name="wqp", bufs=1))
        attn_p = tr_ctx.enter_context(tc.tile_pool(name="attn", bufs=2))
        ot_p = tr_ctx.enter_context(tc.tile_pool(name="otp", bufs=1))
        otb = [ot_p.tile([128, TOK], BF16, tag=f"ot{m}", name=f"otb{m}")
               for m in range(4)]

        def transformer_layer(li, x_in, x_mid):
            wq = [wqp.tile([128, 3 * D], F32R, tag=f"wqkv{k}", name=f"wq{k}")
                  for k in range(4)]
            for k in range(4):
                nc.sync.dma_start(wq[k][:], wqkv_d[li, k * 128:(k + 1) * 128, :])
            wsbq = rows_p.tile([2, 3 * D], F32R, tag="wsbq", name="wsbq", bufs=1)
            nc.sync.dma_start(wsbq[:], wsbq_d[li])
            cols = stat_p.tile([128, 12], F32, tag="tcols", name="tcols")
            nc.sync.dma_start(cols[:], tcols_d[li])

            nmsd = attn_p.tile([2, TOK], F32R, tag="nmsd", name="nmsd", bufs=1)
            rall = attn_p.tile([4, 512], F32R, tag="rall", name="rall", bufs=1)
            rcol = attn_p.tile([128, 16], F32R, tag="rcol", name="rcol", bufs=1)
            ln1_rows(x_in, nmsd, rall)
            rview = rcol[:].rearrange("p (f s) -> p f s", f=4)
            for st in range(4):
                pst4 = pp_bc.tile([128, 4], F32R, tag="bc", name="pst4")
                nc.tensor.transpose(
                    pst4[:], rall[0:4, st * 128:(st + 1) * 128],
                    ident[0:4, 0:4])
                nc.scalar.copy(rview[:, :, st], pst4[:])

            for st in range(4):
                qt = attn_p.tile([128, TOK], BF16, tag="qst", name="qt", bufs=1)
                kt = attn_p.tile([128, TOK], BF16, tag="kst", name="kt", bufs=1)
                vt = attn_p.tile([128, TOK], BF16, tag="vst", name="vt", bufs=1)
                qkv_dst = [qt, kt, vt]
                for f in range(NF):
                    c0 = f * 512 + st * 128
                    for ns in range(3):
                        ps = pp_mm.tile([128, 512], F32, tag="mm", name="qps")
                        for k in range(4):
                            nc.tensor.matmul(
                                ps[:], x_in[k][:, c0:c0 + 128],
                                wq[k][:, ns * 512:(ns + 1) * 512],
                                start=(k == 0), stop=False)
                        nc.tensor.matmul(ps[:], nmsd[0:2, c0:c0 + 128],
                                         wsbq[:, ns * 512:(ns + 1) * 512],
                                         start=False, stop=True)
                        nc.scalar.activation(
                            qkv_dst[ns][:, f * 512:(f + 1) * 512], ps[:],
                            AF.Copy, scale=rcol[:, f * 4 + st:f * 4 + st + 1])

                # --- scores: one batched mul + halving tree over d ---------
                # pbig4 layout (i, j, h, d); tree output s4 layout (i, j, h)
                pbig4 = attn_p.tile([128, 4 * TOK], BF16, tag="bigA",
                                    name="pbig4", bufs=1)
                nc.vector.tensor_mul(
                    pbig4[:].rearrange("p (i j e) -> p i j e", i=4, j=4),
                    qt[:].rearrange("p (i e) -> p i e", i=4).unsqueeze(2)
                    .broadcast_to([128, 4, 4, 512]),
                    kt[:].rearrange("p (j e) -> p j e", j=4).unsqueeze(1)
                    .broadcast_to([128, 4, 4, 512]))
                cA = attn_p.tile([128, 2 * TOK], BF16, tag="midA",
                                 name="cA", bufs=1)
                s4 = attn_p.tile([128, 128], BF16, tag="s_sc", name="s4",
                                 bufs=1)
                with nc.allow_low_precision(reason="qk tree (tiny logits)"):
                    lvl = [(pbig4, 64), (cA, 32), (pbig4, 16), (cA, 8),
                           (pbig4, 4), (cA, 2)]
                    for n, (src, w) in enumerate(lvl):
                        dst, _ = lvl[n + 1] if n + 1 < len(lvl) else (s4, 1)
                        h = w // 2
                        sv = src[:, 0:128 * w].rearrange("p (g d) -> p g d",
                                                         g=128)
                        nc.vector.tensor_tensor(
                            dst[:, 0:128 * h].rearrange("p (g d) -> p g d",
                                                        g=128),
                            sv[:, :, 0:h], sv[:, :, h:w], op=ALU.add)
                # softmax over j without max-subtraction (logits bounded)
                # s4 cols = i*32 + j*8 + h -> exp -> A cols = i*32 + h*4 + j
                eexp = attn_p.tile([128, 128], BF16, tag="eexp", name="eexp",
                                   bufs=1)
                nc.scalar.activation(
                    eexp[:].rearrange("p (i h j) -> p i h j", i=4, h=8),
                    s4[:].rearrange("p (i j h) -> p i h j", i=4, j=4), AF.Exp)
                z = attn_p.tile([128, 32], F32, tag="z", name="zt", bufs=1)
                nc.vector.tensor_reduce(
                    z[:].rearrange("p (i h) -> p i h", i=4),
                    eexp[:].rearrange("p (i h j) -> p i h j", i=4, h=8),
                    axis=AX.X, op=ALU.add)
                zr = attn_p.tile([128, 32], F32, tag="zr", name="zr", bufs=1)
                nc.vector.reciprocal(zr[:], z[:])
                a_t = attn_p.tile([128, 128], BF16, tag="a_t", name="a_t",
                                  bufs=1)
                nc.vector.tensor_mul(
                    a_t[:].rearrange("p (i h j) -> p i h j", i=4, h=8),
                    eexp[:].rearrange("p (i h j) -> p i h j", i=4, h=8),
                    zr[:].rearrange("p (i h) -> p i h", i=4).unsqueeze(3)
                    .broadcast_to([128, 4, 8, 4]))
                # replicate attention weights in pairs so the AV mul keeps
                # packed innermost access (DVE 2x mode)
                arep2 = attn_p.tile([128, 256], BF16, tag="arep2",
                                    name="arep2", bufs=1)
                nc.vector.tensor_copy(
                    arep2[:].rearrange("p (i h j t) -> p i h j t", i=4, h=8,
                                       j=4),
                    a_t[:].rearrange("p (i h j) -> p i h j", i=4, h=8)
                    .unsqueeze(4).broadcast_to([128, 4, 8, 4, 2]))
                # --- AV: per-i muls (2x), then pairwise j-adds -------------
                tbig4 = attn_p.tile([128, 4 * TOK], BF16, tag="bigA",
                                    name="tbig4", bufs=1)
                for i in range(4):
                    nc.vector.tensor_mul(
                        tbig4[:, i * TOK:(i + 1) * TOK]
                        .rearrange("p (j h dp t) -> p j h dp t", j=4, h=8,
                                   dp=32),
                        vt[:].rearrange("p (j h dp t) -> p j h dp t", j=4,
                                        h=8, dp=32),
                        arep2[:, i * 64:(i + 1) * 64]
                        .rearrange("p (h j t) -> p j h t", h=8, j=4)
                        .unsqueeze(3).broadcast_to([128, 4, 8, 32, 2]))
                o2 = attn_p.tile([128, 2 * TOK], BF16, tag="midA", name="o2",
                                 bufs=1)
                with nc.allow_low_precision(reason="av pair add"):
                    nc.vector.tensor_tensor(
                        o2[:].rearrange("p (i e) -> p i e", i=4),
                        tbig4[:].rearrange("p (i e) -> p i e", i=4)[:, :, 0:1024],
                        tbig4[:].rearrange("p (i e) -> p i e", i=4)[:, :, 1024:2048],
                        op=ALU.add)
                of32 = attn_p.tile([128, TOK], F32R, tag="oall", name="of32",
                                   bufs=1)
                with nc.allow_low_precision(reason="f32r out"):
                    nc.vector.tensor_tensor(
                        of32[:].rearrange("p (i e) -> p i e", i=4),
                        o2[:].rearrange("p (i e) -> p i e", i=4)[:, :, 0:512],
                        o2[:].rearrange("p (i e) -> p i e", i=4)[:, :, 512:1024],
                        op=ALU.add)
                for i in range(4):
                    for c in range(4):
                        pst = pp_bc.tile([128, 128], F32R, tag="bc", name="pst")
                        nc.tensor.transpose(
                            pst[:], of32[:, i * 512 + c * 128:
                                         i * 512 + (c + 1) * 128], ident[:])
                        nc.scalar.copy(
                            otb[c][:, i * 512 + st * 128:i * 512 + st * 128 + 128],
                            pst[:])

            wo = [wp.tile([128, D], BF16, tag=f"wo{k}", name=f"wo{k}")
                  for k in range(4)]
            for k in range(4):
                nc.sync.dma_start(wo[k][:], wo_d[li, k * 128:(k + 1) * 128, :])
            for m in range(4):
                for ns in range(4):
                    ps = pp_mm.tile([128, 512], F32, tag="mm", name="ops")
                    for k in range(4):
                        nc.tensor.matmul(
                            ps[:], wo[k][:, m * 128:(m + 1) * 128],
                            otb[k][:, ns * 512:(ns + 1) * 512],
                            start=(k == 0), stop=(k == 3))
                    nc.vector.scalar_tensor_tensor(
                        x_mid[m][:, ns * 512:(ns + 1) * 512], ps[:],
                        cols[:, 4 + m:5 + m], x_in[m][:, ns * 512:(ns + 1) * 512],
                        op0=ALU.add, op1=ALU.add)

            w1 = [wp.tile([128, D], F32R, tag=f"w1_{k}", name=f"w1_{k}")
                  for k in range(4)]
            w2 = [wp.tile([128, D], F32R, tag=f"w2_{k}", name=f"w2_{k}")
                  for k in range(4)]
            for k in range(4):
                nc.sync.dma_start(w1[k][:], w1_d[li, k * 128:(k + 1) * 128, :])
                nc.sync.dma_start(w2[k][:], w2_d[li, k * 128:(k + 1) * 128, :])
            for ns in range(4):
                # scratch aliased onto dead attention buffers (same tags)
                xh2_all = attn_p.tile([128, 2048], F32R, tag="oall",
                                      name="xh2_all", bufs=1)
                xh2 = [xh2_all[:, k * 512:(k + 1) * 512] for k in range(4)]
                ln_cols(x_mid,
                        lambda k: x_mid[k][:, ns * 512:(ns + 1) * 512],
                        xh2, slice(0, 512))
                hsl_all = attn_p.tile([128, 2048], F32R, tag="midA",
                                      name="hsl_all", bufs=1)
                hsl = [hsl_all[:, m * 512:(m + 1) * 512] for m in range(4)]
                for m in range(4):
                    ps = pp_mm.tile([128, 512], F32, tag="mm", name="m1ps")
                    for k in range(4):
                        nc.tensor.matmul(
                            ps[:], w1[k][:, m * 128:(m + 1) * 128], xh2[k][:],
                            start=(k == 0), stop=(k == 3))
                    nc.scalar.activation(hsl[m][:], ps[:], AF.Gelu_apprx_tanh,
                                         bias=cols[:, m:m + 1])
                for m in range(4):
                    ps = pp_mm.tile([128, 512], F32, tag="mm", name="m2ps")
                    for k in range(4):
                        nc.tensor.matmul(
                            ps[:], w2[k][:, m * 128:(m + 1) * 128], hsl[k][:],
                            start=(k == 0), stop=(k == 3))
                    nc.vector.scalar_tensor_tensor(
                        x_mid[m][:, ns * 512:(ns + 1) * 512], ps[:],
                        cols[:, 8 + m:9 + m], x_mid[m][:, ns * 512:(ns + 1) * 512],
                        op0=ALU.add, op1=ALU.add)

        cur, alt = xA, xB
        _nl = int(os.environ.get("KNLAYERS", NLAYER))
        _bar = os.environ.get("KBAR", "0") == "1"
        for li in range(_nl):
            transformer_layer(li, cur, alt)
            cur, alt = alt, cur
            if _bar:
                nc.all_engine_barrier()

        tr_ctx.close()

        # ------------------------------------------------------------------
        # heads
        # ------------------------------------------------------------------
        _skip_heads = os.environ.get("KHEADS", "1") == "0"
        if _skip_heads:
            _da, _db = (int(c) for c in os.environ.get("KDUMP", "01"))
            nc.gpsimd.dma_start(d6_o[:], cur[_da][0:8, :])
            nc.gpsimd.dma_start(tr_o[:], cur[_db][0:8, :])
            for m in range(4):
                nc.gpsimd.dma_start(xdbg_o[m], cur[m][:])
                nc.gpsimd.dma_start(xdbg_o[4 + m], alt[m][:])
        with tc.tile_pool(name="heads", bufs=1) as hp, \
             tc.tile_pool(name="whp", bufs=1) as whp:
          if not _skip_heads:
              hc = stat_p.tile([128, 10], F32, tag="hcols", name="hc")
              nc.sync.dma_start(hc[:], hcols_d[:])
              sb3 = stat_p.tile([8, 2], F32, tag="sb3", name="sb3")
              nc.sync.dma_start(sb3[:], sb3_d[:])

              xhf = [hp.tile([128, TOK], F32R, tag=f"xhf{k}", name=f"xhf{k}")
                     for k in range(4)]
              for st in range(4):
                  ln_cols(cur, lambda k: cur[k][:, st * 512:(st + 1) * 512],
                          xhf, slice(st * 512, (st + 1) * 512))

              xp = alt

              def mm_head(src_tiles, wt_dram, kdim, mdim, dst_tiles, evict):
                  KC = kdim // 128
                  MC = max(mdim // 128, 1)
                  wsb = [whp.tile([128, mdim], F32R, tag=f"wh_{kdim}_{mdim}_{k}",
                                  name=f"wh{k}") for k in range(KC)]
                  for k in range(KC):
                      nc.sync.dma_start(wsb[k][:], wt_dram[k * 128:(k + 1) * 128, :])
                  for m in range(MC):
                      for ns in range(4):
                          ps = pp_mm.tile([128, 512], F32, tag="mm", name="hps")
                          for k in range(KC):
                              nc.tensor.matmul(
                                  ps[:], wsb[k][:, m * 128:(m + 1) * 128],
                                  src_tiles[k][:, ns * 512:(ns + 1) * 512],
                                  start=(k == 0), stop=(k == KC - 1))
                          evict(ps, dst_tiles[m], m, ns)

              mm_head(xhf, projw_d, D, D, xp,
                      lambda ps, dst, m, ns: nc.scalar.activation(
                          dst[:, ns * 512:(ns + 1) * 512], ps[:], AF.Identity,
                          bias=hc[:, m:m + 1]))

              def branch(w1d, w2d, w3d, b1ofs, b2ofs, out_dram, b3col, r1, r2, pfx):
                  mm_head(xp, w1d, D, 256, r1,
                          lambda ps, dst, m, ns: nc.scalar.activation(
                              dst[:, ns * 512:(ns + 1) * 512], ps[:], AF.Relu,
                              bias=hc[:, b1ofs + m:b1ofs + m + 1]))
                  mm_head(r1, w2d, 256, 128, r2,
                          lambda ps, dst, m, ns: nc.scalar.activation(
                              dst[:, ns * 512:(ns + 1) * 512], ps[:], AF.Relu,
                              bias=hc[:, b2ofs:b2ofs + 1]))
                  w3 = whp.tile([128, 8], F32R, tag=f"w3{pfx}", name="w3")
                  nc.sync.dma_start(w3[:], w3d[:])
                  out_sb = hp.tile([8, TOK], F32, tag=f"{pfx}out", name=f"{pfx}out")
                  for ns in range(4):
                      ps = pp_mm.tile([8, 512], F32, tag="mm", name="bps")
                      nc.tensor.matmul(ps[:], w3[:], r2[0][:, ns * 512:(ns + 1) * 512],
                                       start=True, stop=True)
                      nc.scalar.activation(out_sb[:, ns * 512:(ns + 1) * 512], ps[:],
                                           AF.Identity, bias=b3col)
                  nc.sync.dma_start(out_dram[:], out_sb[:])

              # reuse dead transformer buffers for intermediates
              branch(rw1_d, rw2_d, rw3_d, 4, 8, d6_o, sb3[:, 0:1],
                     [cur[0], cur[1]], [xhf[0]], "r")
              branch(tw1_d, tw2_d, tw3_d, 6, 9, tr_o, sb3[:, 1:2],
                     [cur[2], cur[3]], [xhf[1]], "t")

    nc.compile()
    return nc


# ----------------------------------------------------------------------------
# host side
# ----------------------------------------------------------------------------

_CACHE = {}


def _normalize_np(v, eps=1e-12):
    return v / np.maximum(np.linalg.norm(v, axis=-1, keepdims=True), eps)


def _rot6d_np(d6):
    a1, a2 = d6[..., :3], d6[..., 3:]
    b1 = _normalize_np(a1)
    b2 = _normalize_np(a2 - np.sum(b1 * a2, -1, keepdims=True) * b1)
    b3 = np.cross(b1, b2)
    return np.stack([b1, b2, b3], axis=-2)


def _prep_weights(inp):
    f32 = np.float32
    wmap = {}
    for i, cw in enumerate(['c1w', 'c2w', 'c3w', 'c4w']):
        wmap[f'convw{i}'] = np.ascontiguousarray(inp[cw].T.astype(np.float16))
    for i, (g, b2) in enumerate([('bn1g', 'bn1b'), ('bn2g', 'bn2b'),
                                 ('bn3g', 'bn3b')]):
        M = CONV_DIMS[i + 1] // 128
        bn = np.concatenate([
            inp[g].reshape(M, 128).T, inp[b2].reshape(M, 128).T,
            inp[f'c{i + 1}b'].reshape(M, 128).T], axis=1)
        wmap[f'bnconst{i}'] = np.ascontiguousarray(bn.astype(f32))
    pe = _pe_table()[:NF]
    b4 = inp['c4b'][None, :].astype(f32) + pe                   # (4, 512)
    # cols: m*4 + pt ; frame index == pt
    wmap['bias4'] = np.ascontiguousarray(
        b4.reshape(NF, 4, 128).transpose(2, 1, 0).reshape(128, 16).astype(f32))

    qkvw = np.array(inp['qkvw'], f32)
    qkvb = np.array(inp['qkvb'], f32)
    qkvw[:, :, :512] /= math.sqrt(DH)
    qkvb[:, :512] /= math.sqrt(DH)
    g1 = np.array(inp['ln1g'], f32)
    b1 = np.array(inp['ln1b'], f32)
    wq_fold = g1[:, :, None] * qkvw
    bq_fold = qkvb + np.einsum('ld,ldf->lf', b1, qkvw)
    wmap['wqkv'] = np.ascontiguousarray(wq_fold.astype(f32))
    wmap['wsbq'] = np.ascontiguousarray(np.stack(
        [wq_fold.sum(axis=1), bq_fold], axis=1).astype(f32))
    wmap['wo'] = np.ascontiguousarray(np.array(inp['outw'], f32)
                                      .astype(ml_dtypes.bfloat16))
    g2 = np.array(inp['ln2g'], f32)
    bl2 = np.array(inp['ln2b'], f32)
    m1w = np.array(inp['m1w'], f32)
    w1_fold = g2[:, :, None] * m1w
    b1_fold = np.array(inp['m1b'], f32) + np.einsum('ld,ldf->lf', bl2, m1w)
    wmap['w1'] = np.ascontiguousarray(w1_fold.astype(f32))
    wmap['w2'] = np.ascontiguousarray(np.array(inp['m2w'], f32))
    cols = np.zeros((NLAYER, 128, 12), f32)
    cols[:, :, 0:4] = b1_fold.reshape(NLAYER, 4, 128).transpose(0, 2, 1)
    cols[:, :, 4:8] = np.array(inp['outb'], f32).reshape(NLAYER, 4, 128) \
        .transpose(0, 2, 1)
    cols[:, :, 8:12] = np.array(inp['m2b'], f32).reshape(NLAYER, 4, 128) \
        .transpose(0, 2, 1)
    wmap['tcols'] = cols

    gf_ = np.array(inp['lnfg'], f32)
    bf_ = np.array(inp['lnfb'], f32)
    projw = np.array(inp['projw'], f32)
    wmap['projw'] = np.ascontiguousarray(gf_[:, None] * projw)
    projb_fold = np.array(inp['projb'], f32) + bf_ @ projw
    wmap['rw1'] = np.ascontiguousarray(np.array(inp['rw1'], f32))
    wmap['rw2'] = np.ascontiguousarray(np.array(inp['rw2'], f32))
    rw3 = np.zeros((128, 8), f32)
    rw3[:, :6] = np.array(inp['rw3'], f32)
    wmap['rw3'] = rw3
    wmap['tw1'] = np.ascontiguousarray(np.array(inp['tw1'], f32))
    wmap['tw2'] = np.ascontiguousarray(np.array(inp['tw2'], f32))
    tw3 = np.zeros((128, 8), f32)
    tw3[:, :3] = np.array(inp['tw3'], f32)
    wmap['tw3'] = tw3
    hcols = np.zeros((128, 10), f32)
    hcols[:, 0:4] = projb_fold.reshape(4, 128).T
    hcols[:, 4:6] = np.array(inp['rb1'], f32).reshape(2, 128).T
    hcols[:, 6:8] = np.array(inp['tb1'], f32).reshape(2, 128).T
    hcols[:, 8] = np.array(inp['rb2'], f32)
    hcols[:, 9] = np.array(inp['tb2'], f32)
    wmap['hcols'] = hcols
    sb3 = np.zeros((8, 2), f32)
    sb3[0:6, 0] = np.array(inp['rb3'], f32)
    sb3[0:3, 1] = np.array(inp['tb3'], f32)
    wmap['sb3'] = sb3
    wmap['ones_c'] = np.ones((128, 128), f32)
    return wmap


def kernel(**inputs):
    inp = {k: np.asarray(v) for k, v in inputs.items()}

    idx = inp['seed_idxs'].reshape(B, -1).astype(np.int64)      # (B, N)
    sel_seed = np.take_along_axis(np.asarray(inp['fp2_features'], np.float32),
                                  idx[:, None, :], axis=2)
    sel_grasp = np.take_along_axis(np.asarray(inp['local_grasp_features'], np.float32),
                                   idx[:, None, :], axis=2)
    sel_color = np.take_along_axis(np.asarray(inp['local_color_features'], np.float32),
                                   idx[:, None, :], axis=2)
    sel_pose = np.take_along_axis(np.asarray(inp['grasp_pose_feature'], np.float32),
                                  idx[:, None, :], axis=2)
    gsf = np.asarray(inp['sa4_features'], np.float32).max(axis=-1)
    gsf = np.broadcast_to(gsf[:, :, None], (B, 256, NPTS))
    fused = sel_pose + np.concatenate([sel_grasp, sel_color, sel_seed, gsf], axis=1)
    gf = fused.reshape(BE, FRAME, 1024, NPTS)
    cond = np.broadcast_to(gf[:, :1], (BE, NF, 1024, NPTS))
    X = np.concatenate([cond, gf[:, 1:]], axis=2)               # (e, f, 2048, N)

    if 'nc' not in _CACHE:
        _CACHE['nc'] = build_kernel()
    nc = _CACHE['nc']
    wmap = _prep_weights(inp)

    in_maps = []
    for k in range(NCORES):
        xc = X[:, :, :, k * NPC:(k + 1) * NPC]                  # (e, f, c, n)
        xc = xc.transpose(2, 1, 0, 3).reshape(2048, TOK)        # (c, (f,e,n))
        m = dict(wmap)
        m['xin'] = np.ascontiguousarray(xc, dtype=np.float16)
        in_maps.append(m)

    res = run_bass_kernel_spmd(nc, in_maps, core_ids=list(range(NCORES)))
    global _LAST_RES
    _LAST_RES = res

    out = np.zeros((BE * NPTS, NF, 12), np.float32)
    for k in range(NCORES):
        d6 = res.results[k]['d6'][:6]
        tr = res.results[k]['tr3'][:3]
        d6 = d6.reshape(6, NF, BE, NPC).transpose(2, 3, 1, 0)   # (e, n, f, 6)
        tr = tr.reshape(3, NF, BE, NPC).transpose(2, 3, 1, 0)
        rot = _rot6d_np(d6).reshape(BE, NPC, NF, 9)
        for e in range(BE):
            rows = slice(e * NPTS + k * NPC, e * NPTS + (k + 1) * NPC)
            out[rows, :, 0:3] = tr[e]
            out[rows, :, 3:12] = rot[e]
    return out


if __name__ == "__main__":
    build_kernel()
    print("built ok")

